# revision 12
# baseline (speedup 1.0000x reference)
"""Trainium2 Bass kernel for nn_ModalGenerator (MoE-routed cross-modal generator).

Strategy (v2):
  - seq_len==1 => attention collapses to v = tgt @ wv.T; fold wv/ao_w into one
    512x512 matrix per layer (host-side) and (1-rw) into the output projection.
  - MoE routing on host: gather missing_type==1 columns (gen0) and ==2 (gen1);
    missing_type==3 rows use the tiny host-computed prior MLP.
  - Generator-split sharding: cores 0-3 run generator 0 on 1/4 of its columns
    each, cores 4-7 run generator 1. Zero collectives (host gathers/scatters).
  - All projections run in fp8-e4m3 DoubleRow mode (K=256 per pass, 2x PE
    rate): weights scaled x64, the 1/64 unscale folded into downstream scales.
  - Zero-mean-by-construction LN (requires gamma==1, beta==0, zero biases,
    which setup_inputs always produces): LN output is exactly zero-mean, and
    every pre-LN activation is (centered GEMM) + (previous LN output), so
    centering ip/wa/f2 weights over their output dim on the host makes every
    pre-LN activation zero-mean. This removes the mean stats matmuls, the
    mean subtraction, and the identity-residual matmuls of v1 entirely:
      * residual adds ride the PSUM->SBUF copy (DVE scalar_tensor_tensor)
      * LN = sumsq stat (4 bf16 ones-matmuls) -> rstd via bf16 bit-hack with
        the 1/H folded into the magic constant (reads the high half-word of
        the f32 PSUM stat directly) -> single y*rstd apply.
  - Engine balance: PE GEMMs+stats; DVE psum copies/residuals, ysq, shift;
    Pool (gpsimd) magic op + LN apply muls (SBUF only - gpsimd cannot read
    PSUM); ACT gelu + fp8 copies of LN1 outputs + final-LN fp8.
  - Wavefront software pipelining across NT=5 column tiles (<=256 wide),
    10 stages per layer + out stage; later stages issue first within a wave.
  - PSUM: 3 "mm" buffers ([128,4,256] = 2 banks each) + 2 "st" single-bank
    stat buffers = 8 banks.
  - Fallback: if the instance has nonzero biases / non-unit gamma / nonzero
    beta, use the v1 general program (kept verbatim below).
"""

import math

import numpy as np
import ml_dtypes

import concourse.bacc as bacc
import concourse.mybir as mybir
import concourse.tile as tile
from concourse.bass_utils import run_bass_kernel_spmd

f32 = mybir.dt.float32
bf16 = mybir.dt.bfloat16
f8 = mybir.dt.float8e4
i16 = mybir.dt.int16
AF = mybir.ActivationFunctionType
ALU = mybir.AluOpType
DR = mybir.MatmulPerfMode.DoubleRow

H = 512
L = 3
N_CORES = 8
GCORES = 4               # cores per generator
KC = H // 128            # 4 k-chunks of the hidden dim
FH = 4 * H               # 2048 FFN hidden
FKC = FH // 128          # 16
LN_EPS = 1e-5
MAGIC16 = 0x5F37
MAGIC16_H = 0x5F37 + 576  # folds the 1/H (H=512=2^9) into the bit-hack
W8SCALE = 64.0           # fp8 weight pre-scale

FFN_FP8 = True

# param pack column layout: [128, 128] f32 (general path only)
_P_IPB = 0
_P_LAYER = 4             # + 40*l: ba 0..3 | f1b 4..19 | f2b 20..23
#                                 | ln1g 24..27 | ln1b 28..31 | ln2g 32..35 | ln2b 36..39
_P_OPB = 124


def _pack_pcol(vec):
    """[n*128] vector -> [128, n] chunk-column layout."""
    return np.ascontiguousarray(np.asarray(vec, np.float32).reshape(-1, 128).T)


def _sb_pack(wT, dt):
    """[K, M] (K mult of 128) -> [128, (K/128)*M] SBUF chunk-major layout."""
    K, M = wT.shape
    a = np.asarray(wT, np.float32).astype(dt)
    return np.ascontiguousarray(
        a.reshape(K // 128, 128, M).transpose(1, 0, 2).reshape(128, -1))


NT_TARGET = 6            # pipeline depth (equal column tiles per core)


def _tiles(C):
    nt = min(NT_TARGET, max(1, C // 64))
    base = C // nt // 16 * 16
    sizes = [base] * nt
    extra = C - base * nt
    i = 0
    while extra > 0:
        sizes[i] += min(16, extra)
        extra -= 16
        i = (i + 1) % nt
    # small first and last tiles (short serial pipeline fill and drain),
    # big middle tiles; cap 256 so 4 psum chunks pack into 2 banks
    sizes.sort(reverse=True)
    if nt >= 3:
        ends = [sizes.pop(), sizes.pop()]
        for i in range(2):
            for _ in range(7):
                j = sizes.index(min(sizes))
                if ends[i] - 16 >= 64 and sizes[j] + 16 <= 256:
                    ends[i] -= 16
                    sizes[j] += 16
        sizes.sort(reverse=True)
        sizes = [ends[0]] + sizes + [ends[1]]
    assert max(sizes) <= 256
    ts = []
    c0 = 0
    for s in sizes:
        if s > 0:
            ts.append((c0, c0 + s))
            c0 += s
    assert c0 == C
    return ts


def _build_program_v2(C):
    """Fast path: zero biases, unit gamma, zero beta (always true for the
    harness inputs). Weights ip/wa/f2 are centered host-side."""
    nc = bacc.Bacc("TRN2", target_bir_lowering=False, debug=False,
                   num_devices=N_CORES)

    dram = {
        "src": nc.dram_tensor("src", [128, KC * C], f8, kind="ExternalInput"),
        "tgt": nc.dram_tensor("tgt", [128, KC * C], f8, kind="ExternalInput"),
        "ip": nc.dram_tensor("ip", [128, KC * H], f8, kind="ExternalInput"),
        "op": nc.dram_tensor("op", [128, KC * H], f8, kind="ExternalInput"),
        "wa": nc.dram_tensor("wa", [L, 128, KC * H], f8, kind="ExternalInput"),
        "f1": nc.dram_tensor("f1", [L, 128, KC * FH], f8, kind="ExternalInput"),
        "f2": nc.dram_tensor("f2", [L, 128, FKC * H], f8, kind="ExternalInput"),
        "ones": nc.dram_tensor("ones", [128, 128], bf16, kind="ExternalInput"),
        "id8": nc.dram_tensor("id8", [128, 3 * 128], f8, kind="ExternalInput"),
        "out": nc.dram_tensor("out", [128, KC * C], bf16, kind="ExternalOutput"),
    }
    tiles = _tiles(C)
    NT = len(tiles)
    unsc = 1.0 / W8SCALE

    with tile.TileContext(nc) as tc:
        with (
            tc.tile_pool(name="sb", bufs=2) as sb,
            tc.tile_pool(name="ps", bufs=2, space="PSUM") as psp,
        ):
            ones = sb.tile([128, 128], bf16, tag="ones", bufs=1)
            nc.sync.dma_start(ones[:], dram["ones"].ap())
            id8 = sb.tile([128, 3, 128], f8, tag="id8", bufs=1)
            nc.sync.dma_start(id8[:], dram["id8"].ap().rearrange(
                "p (a b) -> p a b", a=3))
            ipw = sb.tile([128, KC, H], f8, tag="ip", bufs=1)
            nc.sync.dma_start(ipw[:], dram["ip"].ap())
            wa0 = sb.tile([128, KC, H], f8, tag="wa", bufs=2)
            nc.sync.dma_start(wa0[:], dram["wa"].ap()[0])
            srcT = sb.tile([128, KC * C], f8, tag="src", bufs=1)
            tgtT = sb.tile([128, KC * C], f8, tag="tgt", bufs=1)
            nc.sync.dma_start(srcT[:, 0:KC * tiles[0][1]],
                              dram["src"].ap()[:, 0:KC * tiles[0][1]])
            nc.sync.dma_start(tgtT[:, 0:KC * tiles[0][1]],
                              dram["tgt"].ap()[:, 0:KC * tiles[0][1]])
            # PE p-state warmup: keep the tensor engine continuously busy
            # from ~1us so real GEMMs start at the full 2.4 GHz clock, and
            # preload the ACT gelu/copy table during the DMA fill.
            warm = psp.tile([128, 512], f32, tag="st", bufs=2)
            for wi in range(30):
                nc.tensor.matmul(warm[:, 0:128], ones[:], ones[:],
                                 start=(wi == 0), stop=(wi == 29),
                                 skip_group_check=True)
            wact = sb.tile([128, 128], bf16, tag="wact", bufs=1)
            nc.scalar.activation(wact[:], ones[:], AF.Gelu)
            for ti in range(1, NT):
                c0, c1 = tiles[ti]
                nc.sync.dma_start(srcT[:, KC * c0:KC * c1],
                                  dram["src"].ap()[:, KC * c0:KC * c1])
                nc.sync.dma_start(tgtT[:, KC * c0:KC * c1],
                                  dram["tgt"].ap()[:, KC * c0:KC * c1])

            def _tm(flat, ti, p):
                c0, c1 = tiles[ti]
                Ct = c1 - c0
                sl = flat[:, KC * c0 + 2 * p * Ct:KC * c0 + (2 * p + 2) * Ct]
                return sl.rearrange("q (a b) -> q a b", a=2)

            was, f1s, f2s = [wa0], [], []
            for l in range(L):
                if l > 0:
                    wa = sb.tile([128, KC, H], f8, tag="wa", bufs=2)
                    nc.sync.dma_start(wa[:], dram["wa"].ap()[l])
                    was.append(wa)
                f1w = sb.tile([128, KC, FH], f8, tag="f1", bufs=2)
                nc.sync.dma_start(f1w[:], dram["f1"].ap()[l])
                f1s.append(f1w)
                f2w = sb.tile([128, FKC, H], f8, tag="f2", bufs=2)
                nc.sync.dma_start(f2w[:], dram["f2"].ap()[l])
                f2s.append(f2w)
            opw = sb.tile([128, KC, H], f8, tag="op", bufs=1)
            nc.sync.dma_start(opw[:], dram["op"].ap())

            # per-layer activation tensors (full C width, per-tile writes)
            xn_all = []          # ln2_out bf16 per layer (attn residual)
            xf8_all = []         # ln1_out fp8 (f1 input + f2 residual);
            #                      [L-1] reused for op input
            for l in range(L):
                xn_b = sb.tile([128, KC, C], bf16, tag="x", bufs=2)
                xn_all.append((None, xn_b))
                xf8_t = sb.tile([128, KC, C], f8, tag="xf8", bufs=2)
                xf8_all.append(xf8_t)

            ps_all = [[None] * NT for _ in range(L)]   # attn psum
            pf_all = [[None] * NT for _ in range(L)]   # f2 psum
            y1_all = [[None] * NT for _ in range(L)]
            y2_all = [[None] * NT for _ in range(L)]
            st1_all = [[None] * NT for _ in range(L)]
            st2_all = [[None] * NT for _ in range(L)]
            rs1_all = [[None] * NT for _ in range(L)]
            rs2_all = [[None] * NT for _ in range(L)]
            hh_all = [[None] * NT for _ in range(L)]

            def _attn_mm(l, ti):
                wa = was[l]
                c0, c1 = tiles[ti]
                Ct = c1 - c0
                ps = psp.tile([128, 4, 256], f32, tag="mm", bufs=3)
                ps_all[l][ti] = ps
                for j in range(KC):
                    first = (j % 2 == 0)
                    last = (j % 2 == 1)
                    if l == 0:
                        for p in range(KC // 2):
                            nc.tensor.matmul(
                                ps[:, j, 0:Ct],
                                ipw[:, 2 * p:2 * p + 2, 128 * j:128 * (j + 1)],
                                _tm(srcT, ti, p),
                                start=(p == 0 and first), stop=False,
                                perf_mode=DR, skip_group_check=True)
                        for p in range(KC // 2):
                            nc.tensor.matmul(
                                ps[:, j, 0:Ct],
                                wa[:, 2 * p:2 * p + 2, 128 * j:128 * (j + 1)],
                                _tm(tgtT, ti, p),
                                start=False,
                                stop=(p == KC // 2 - 1 and last),
                                perf_mode=DR, skip_group_check=True)
                    else:
                        for p in range(KC // 2):
                            nc.tensor.matmul(
                                ps[:, j, 0:Ct],
                                wa[:, 2 * p:2 * p + 2, 128 * j:128 * (j + 1)],
                                _tm(tgtT, ti, p),
                                start=(p == 0 and first),
                                stop=(p == KC // 2 - 1 and last),
                                perf_mode=DR, skip_group_check=True)

            def _y1(l, ti):
                c0, c1 = tiles[ti]
                Ct = c1 - c0
                ps = ps_all[l][ti]
                y1 = sb.tile([128, KC, 256], bf16, tag="y", bufs=2 * NT)
                y1_all[l][ti] = y1
                if l == 0:
                    nc.vector.tensor_scalar(y1[:, :, 0:Ct], ps[:, :, 0:Ct],
                                            unsc, None, ALU.mult)
                else:
                    xp = xn_all[l - 1][1]
                    nc.vector.scalar_tensor_tensor(
                        y1[:, :, 0:Ct], ps[:, :, 0:Ct], unsc,
                        xp[:, :, c0:c1], ALU.mult, ALU.add)

            def _sq(y, Ct, holder, l, ti):
                """ysq -> sumsq stat -> shifted high half-words (the stat is
                consumed here so its PSUM slot frees within the stage)."""
                ysq = sb.tile([128, KC, 256], bf16, tag="ysq", bufs=3)
                nc.vector.tensor_mul(ysq[:, :, 0:Ct], y[:, :, 0:Ct],
                                     y[:, :, 0:Ct])
                st = psp.tile([128, 512], f32, tag="st", bufs=2)
                for k in range(KC):
                    nc.tensor.matmul(st[:, 0:Ct], ones[:], ysq[:, k, 0:Ct],
                                     start=(k == 0), stop=(k == KC - 1),
                                     skip_group_check=True)
                sh = sb.tile([128, 256], i16, tag="sh", bufs=4)
                st16 = st.bitcast(i16)          # [128, 1024]
                nc.vector.tensor_scalar(sh[:, 0:Ct], st16[:, 1:2 * Ct:2],
                                        1, None, ALU.logical_shift_right)
                holder[l][ti] = sh

            def _rstd(sh, Ct):
                """rstd = magic - (bits >> 1), 1/H folded into the magic."""
                rstd = sb.tile([128, 256], bf16, tag="rstd", bufs=4)
                nc.vector.tensor_scalar(rstd[:, 0:Ct].bitcast(i16),
                                        sh[:, 0:Ct], -1, MAGIC16_H,
                                        ALU.mult, ALU.add)
                return rstd

            def s1_stage(l, ti):
                _attn_mm(l, ti)

            def s1b_stage(l, ti):
                _y1(l, ti)

            def s2_stage(l, ti):
                c0, c1 = tiles[ti]
                _sq(y1_all[l][ti], c1 - c0, st1_all, l, ti)

            def s3_stage(l, ti):
                c0, c1 = tiles[ti]
                Ct = c1 - c0
                y1 = y1_all[l][ti]
                rstd = _rstd(st1_all[l][ti], Ct)
                rs1_all[l][ti] = rstd
                r4 = rstd[:, 0:Ct].unsqueeze(1).broadcast_to((128, KC, Ct))
                nc.gpsimd.tensor_mul(xf8_all[l][:, :, c0:c1],
                                     y1[:, :, 0:Ct], r4)

            def f1a_stage(l, ti):
                _f1_half(l, ti, 0)

            def _f1_half(l, ti, half):
                f1w = f1s[l]
                xin = xf8_all[l]
                c0, c1 = tiles[ti]
                Ct = c1 - c0
                if half == 0:
                    hh = sb.tile([128, FKC, 512], f8, tag="h", bufs=2)
                    hh_all[l][ti] = hh
                hh = hh_all[l][ti]
                for g in range(2 * half, 2 * half + 2):
                    ps = psp.tile([128, 4, 256], f32, tag="mm", bufs=3)
                    for j in range(KC):
                        mi = KC * g + j
                        first = (j % 2 == 0)
                        last = (j % 2 == 1)
                        for p in range(KC // 2):
                            nc.tensor.matmul(
                                ps[:, j, 0:Ct],
                                f1w[:, 2 * p:2 * p + 2,
                                    128 * mi:128 * (mi + 1)],
                                xin[:, 2 * p:2 * p + 2, c0:c1],
                                start=(p == 0 and first),
                                stop=(p == KC // 2 - 1 and last),
                                perf_mode=DR, skip_group_check=True)
                    nc.scalar.activation(
                        hh[:, KC * g:KC * g + KC, 0:Ct], ps[:, :, 0:Ct],
                        AF.Gelu, scale=unsc)

            def f1b_stage(l, ti):
                _f1_half(l, ti, 1)

            def f2_stage(l, ti):
                f2w = f2s[l]
                hh = hh_all[l][ti]
                xf8 = xf8_all[l]
                c0, c1 = tiles[ti]
                Ct = c1 - c0
                ps = psp.tile([128, 4, 256], f32, tag="mm", bufs=3)
                pf_all[l][ti] = ps
                for j in range(KC):
                    first = (j % 2 == 0)
                    last = (j % 2 == 1)
                    for p in range(FKC // 2):
                        nc.tensor.matmul(
                            ps[:, j, 0:Ct],
                            f2w[:, 2 * p:2 * p + 2, 128 * j:128 * (j + 1)],
                            hh[:, 2 * p:2 * p + 2, 0:Ct],
                            start=(p == 0 and first), stop=False,
                            perf_mode=DR, skip_group_check=True)
                    # residual: 64*I (fp8 DR) on the LN1 fp8 output
                    pr = j // 2
                    nc.tensor.matmul(
                        ps[:, j, 0:Ct],
                        id8[:, (0 if j % 2 == 0 else 1):(2 if j % 2 == 0 else 3), :],
                        xf8[:, 2 * pr:2 * pr + 2, c0:c1],
                        start=False, stop=last,
                        perf_mode=DR, skip_group_check=True)

            def y2_stage(l, ti):
                c0, c1 = tiles[ti]
                Ct = c1 - c0
                ps = pf_all[l][ti]
                y2 = sb.tile([128, KC, 256], bf16, tag="y", bufs=2 * NT)
                y2_all[l][ti] = y2
                nc.vector.tensor_scalar(y2[:, :, 0:Ct], ps[:, :, 0:Ct],
                                        unsc, None, ALU.mult)

            def sq2_stage(l, ti):
                c0, c1 = tiles[ti]
                _sq(y2_all[l][ti], c1 - c0, st2_all, l, ti)

            def ap2_stage(l, ti):
                c0, c1 = tiles[ti]
                Ct = c1 - c0
                y2 = y2_all[l][ti]
                rstd = _rstd(st2_all[l][ti], Ct)
                rs2_all[l][ti] = rstd
                r4 = rstd[:, 0:Ct].unsqueeze(1).broadcast_to((128, KC, Ct))
                if l < L - 1:
                    xn2 = xn_all[l][1]
                    nc.gpsimd.tensor_mul(xn2[:, :, c0:c1],
                                         y2[:, :, 0:Ct], r4)
                else:
                    # final LN feeds only the output projection: fp8
                    nc.vector.tensor_mul(xf8_all[L - 1][:, :, c0:c1],
                                         y2[:, :, 0:Ct], r4)

            def out_stage(l, ti):
                xo = xf8_all[L - 1]
                c0, c1 = tiles[ti]
                Ct = c1 - c0
                ps = psp.tile([128, 4, 256], f32, tag="mm", bufs=3)
                for j in range(KC):
                    first = (j % 2 == 0)
                    last = (j % 2 == 1)
                    for p in range(KC // 2):
                        nc.tensor.matmul(
                            ps[:, j, 0:Ct],
                            opw[:, 2 * p:2 * p + 2, 128 * j:128 * (j + 1)],
                            xo[:, 2 * p:2 * p + 2, c0:c1],
                            start=(p == 0 and first),
                            stop=(p == KC // 2 - 1 and last),
                            perf_mode=DR, skip_group_check=True)
                ot = sb.tile([128, KC, 512], bf16, tag="o", bufs=2)
                nc.scalar.activation(ot[:, :, 0:Ct], ps[:, :, 0:Ct],
                                     AF.Copy, scale=unsc)
                oap = dram["out"].ap().rearrange("p (a b) -> p a b", a=KC)
                nc.sync.dma_start(oap[:, :, c0:c1], ot[:, :, 0:Ct])

            # wavefront issue: stage ls of tile t at wave ls + t
            stage_fns = []
            for l in range(L):
                stage_fns += [
                    (s1_stage, l), (s1b_stage, l), (s2_stage, l),
                    (s3_stage, l), (f1a_stage, l), (f1b_stage, l),
                    (f2_stage, l), (y2_stage, l), (sq2_stage, l),
                    (ap2_stage, l),
                ]
            stage_fns.append((out_stage, L - 1))
            NS = len(stage_fns)
            # tiles spaced SPREAD waves apart: the active stage window spans
            # SPREAD*NT stages, covering a whole layer cycle with NT=5
            SPREAD = 2
            for wave in range(NS + SPREAD * (NT - 1) + 1):
                for t in range(0, NT):
                    s = wave - SPREAD * t
                    if 0 <= s < NS:
                        fn, l = stage_fns[s]
                        fn(l, t)

    nc.compile()
    return nc


def _build_program_general(C, skips):
    """v1 general program (biases / gamma / beta supported). Verbatim from
    the baseline kernel."""
    zb, ug, zbeta = skips
    f8dt = f8 if FFN_FP8 else bf16
    nc = bacc.Bacc("TRN2", target_bir_lowering=False, debug=False,
                   num_devices=N_CORES)

    dram = {
        "src": nc.dram_tensor("src", [128, KC * C], f8, kind="ExternalInput"),
        "tgt": nc.dram_tensor("tgt", [128, KC * C], f8, kind="ExternalInput"),
        "ip": nc.dram_tensor("ip", [128, KC * H], f8, kind="ExternalInput"),
        "op": nc.dram_tensor("op", [128, KC * H], f8, kind="ExternalInput"),
        "wa": nc.dram_tensor("wa", [L, 128, KC * H], f8, kind="ExternalInput"),
        "f1": nc.dram_tensor("f1", [L, 128, KC * FH], f8dt, kind="ExternalInput"),
        "f2": nc.dram_tensor("f2", [L, 128, FKC * H], f8dt, kind="ExternalInput"),
        "par": nc.dram_tensor("par", [128, 128], f32, kind="ExternalInput"),
        "ones": nc.dram_tensor("ones", [128, 128], bf16, kind="ExternalInput"),
        "ident": nc.dram_tensor("ident", [128, 128], bf16, kind="ExternalInput"),
        "out": nc.dram_tensor("out", [128, KC * C], bf16, kind="ExternalOutput"),
    }
    tiles = _tiles(C)
    NT = len(tiles)
    unsc = 1.0 / W8SCALE if FFN_FP8 else 1.0

    with tile.TileContext(nc) as tc:
        with (
            tc.tile_pool(name="sb", bufs=2) as sb,
            tc.tile_pool(name="ps", bufs=2, space="PSUM") as psp,
        ):
            ipw = sb.tile([128, KC, H], f8, tag="ip", bufs=1)
            nc.sync.dma_start(ipw[:], dram["ip"].ap())
            wa0 = sb.tile([128, KC, H], f8, tag="wa", bufs=2)
            nc.sync.dma_start(wa0[:], dram["wa"].ap()[0])
            srcT = sb.tile([128, KC * C], f8, tag="src", bufs=1)
            tgtT = sb.tile([128, KC * C], f8, tag="tgt", bufs=1)
            nc.sync.dma_start(srcT[:, 0:KC * tiles[0][1]],
                              dram["src"].ap()[:, 0:KC * tiles[0][1]])
            nc.sync.dma_start(tgtT[:, 0:KC * tiles[0][1]],
                              dram["tgt"].ap()[:, 0:KC * tiles[0][1]])
            ones = sb.tile([128, 128], bf16, tag="ones", bufs=1)
            nc.sync.dma_start(ones[:], dram["ones"].ap())
            ident = sb.tile([128, 128], bf16, tag="ident", bufs=1)
            nc.sync.dma_start(ident[:], dram["ident"].ap())
            for ti in range(1, NT):
                c0, c1 = tiles[ti]
                nc.sync.dma_start(srcT[:, KC * c0:KC * c1],
                                  dram["src"].ap()[:, KC * c0:KC * c1])
                nc.sync.dma_start(tgtT[:, KC * c0:KC * c1],
                                  dram["tgt"].ap()[:, KC * c0:KC * c1])

            def _tm(flat, ti, p):
                c0, c1 = tiles[ti]
                Ct = c1 - c0
                sl = flat[:, KC * c0 + 2 * p * Ct:KC * c0 + (2 * p + 2) * Ct]
                return sl.rearrange("q (a b) -> q a b", a=2)
            par = sb.tile([128, 128], f32, tag="par", bufs=1)
            nc.sync.dma_start(par[:], dram["par"].ap())

            def stats_stage(y, Ct):
                ysq = sb.tile([128, KC, 512], bf16, tag="ysq", bufs=2)
                nc.vector.tensor_mul(ysq[:, :, 0:Ct], y[:, :, 0:Ct],
                                     y[:, :, 0:Ct])
                st = psp.tile([128, 2, 256], f32, tag="st", bufs=2)
                for k in range(KC):
                    nc.tensor.matmul(st[:, 0, 0:Ct], ones[:], y[:, k, 0:Ct],
                                     start=(k == 0), stop=False,
                                     skip_group_check=True)
                for k in range(KC):
                    nc.tensor.matmul(st[:, 1, 0:Ct], ones[:], ysq[:, k, 0:Ct],
                                     start=False, stop=(k == KC - 1),
                                     skip_group_check=True)
                return st

            def ln_chain(y, st, Ct, gcol, bcol, xn, xf8, t, newton=True):
                c0, c1 = t
                m = sb.tile([128, 512], bf16, tag="m", bufs=2)
                nc.vector.tensor_scalar(m[:, 0:Ct], st[:, 0, 0:Ct], 1.0 / H,
                                        None, ALU.mult)
                msq = sb.tile([128, 512], bf16, tag="msq", bufs=2)
                nc.vector.scalar_tensor_tensor(msq[:, 0:Ct], st[:, 0, 0:Ct],
                                               1.0 / H, m[:, 0:Ct],
                                               ALU.mult, ALU.mult)
                z = sb.tile([128, 512], bf16, tag="z", bufs=2)
                nc.vector.scalar_tensor_tensor(z[:, 0:Ct], st[:, 1, 0:Ct],
                                               1.0 / H, msq[:, 0:Ct],
                                               ALU.mult, ALU.subtract)
                r = sb.tile([128, 512], bf16, tag="rx", bufs=2)
                nc.vector.tensor_scalar(r[:, 0:Ct].bitcast(i16),
                                        z[:, 0:Ct].bitcast(i16), 1, None,
                                        ALU.logical_shift_right)
                rstd = sb.tile([128, 512], bf16, tag="rstd", bufs=4)
                nc.vector.tensor_scalar(rstd[:, 0:Ct].bitcast(i16),
                                        r[:, 0:Ct].bitcast(i16), -1,
                                        MAGIC16, ALU.mult, ALU.add)
                if newton:
                    u = sb.tile([128, 512], bf16, tag="u", bufs=2)
                    nc.vector.tensor_mul(u[:, 0:Ct], rstd[:, 0:Ct],
                                         rstd[:, 0:Ct])
                    w = sb.tile([128, 512], bf16, tag="w", bufs=2)
                    nc.vector.scalar_tensor_tensor(w[:, 0:Ct], u[:, 0:Ct],
                                                   -0.5, z[:, 0:Ct],
                                                   ALU.mult, ALU.mult)
                    rstd2 = sb.tile([128, 512], bf16, tag="rstd", bufs=4)
                    nc.vector.scalar_tensor_tensor(rstd2[:, 0:Ct], w[:, 0:Ct],
                                                   1.5, rstd[:, 0:Ct],
                                                   ALU.add, ALU.mult)
                else:
                    rstd2 = rstd
                m4 = m[:, 0:Ct].unsqueeze(1).broadcast_to((128, KC, Ct))
                r4 = rstd2[:, 0:Ct].unsqueeze(1).broadcast_to((128, KC, Ct))
                if ug and zbeta:
                    u1 = sb.tile([128, KC, 512], bf16, tag="u1", bufs=2)
                    nc.vector.tensor_sub(u1[:, :, 0:Ct], y[:, :, 0:Ct], m4)
                    if xf8 is not None:
                        nc.gpsimd.tensor_mul(xf8[:, :, c0:c1],
                                             u1[:, :, 0:Ct], r4)
                        nc.vector.tensor_mul(xn[:, :, c0:c1],
                                             u1[:, :, 0:Ct], r4)
                    else:
                        nc.vector.tensor_mul(xn[:, :, c0:c1],
                                             u1[:, :, 0:Ct], r4)
                else:
                    for mm in range(KC):
                        u1 = sb.tile([128, 512], bf16, tag="u1c", bufs=1)
                        nc.vector.tensor_sub(u1[:, 0:Ct], y[:, mm, 0:Ct],
                                             m[:, 0:Ct])
                        u2 = sb.tile([128, 512], bf16, tag="u2c", bufs=1)
                        nc.vector.scalar_tensor_tensor(
                            u2[:, 0:Ct], u1[:, 0:Ct],
                            par[:, gcol + mm:gcol + mm + 1], rstd2[:, 0:Ct],
                            ALU.mult, ALU.mult)
                        nc.vector.tensor_scalar(
                            xn[:, mm, c0:c1], u2[:, 0:Ct],
                            par[:, bcol + mm:bcol + mm + 1], None, ALU.add)
                    if xf8 is not None:
                        nc.scalar.activation(xf8[:, :, c0:c1], xn[:, :, c0:c1],
                                             AF.Copy)

            x = None
            xn_all, xf8_all = [], []
            for l in range(L):
                xn_a = sb.tile([128, KC, C], bf16, tag="x", bufs=3)
                xn_b = sb.tile([128, KC, C], bf16, tag="x", bufs=3)
                xn_all.append((xn_a, xn_b))
                if FFN_FP8:
                    xf8_t = sb.tile([128, KC, C], f8, tag="xf8", bufs=2)
                    xf8_all.append(xf8_t)
                else:
                    xf8_all.append(None)

            was, f1s, f2s = [wa0], [], []
            for l in range(L):
                if l > 0:
                    wa = sb.tile([128, KC, H], f8, tag="wa", bufs=2)
                    nc.sync.dma_start(wa[:], dram["wa"].ap()[l])
                    was.append(wa)
                f1w = sb.tile([128, KC, FH], f8dt, tag="f1", bufs=2)
                nc.sync.dma_start(f1w[:], dram["f1"].ap()[l])
                f1s.append(f1w)
                f2w = sb.tile([128, FKC, H], f8dt, tag="f2", bufs=2)
                nc.sync.dma_start(f2w[:], dram["f2"].ap()[l])
                f2s.append(f2w)
            opw = sb.tile([128, KC, H], f8, tag="op", bufs=1)
            nc.sync.dma_start(opw[:], dram["op"].ap())

            ys_all = [[None] * NT for _ in range(L)]
            y2s_all = [[None] * NT for _ in range(L)]
            hh_all = [[None] * NT for _ in range(L)]

            def attn_stage(l, ti):
                pb = _P_LAYER + 40 * l
                wa = was[l]
                xp = xn_all[l - 1][1] if l > 0 else None
                c0, c1 = tiles[ti]
                Ct = c1 - c0
                y = sb.tile([128, KC, 512], bf16, tag="y", bufs=NT)
                ps = psp.tile([128, 4, 256], f32, tag="mm", bufs=3)
                for j in range(KC):
                    first = (j % 2 == 0)
                    last = (j % 2 == 1)
                    if l == 0:
                        for p in range(KC // 2):
                            nc.tensor.matmul(
                                ps[:, j, 0:Ct],
                                ipw[:, 2 * p:2 * p + 2, 128 * j:128 * (j + 1)],
                                _tm(srcT, ti, p),
                                start=(p == 0 and first), stop=False,
                                perf_mode=DR, skip_group_check=True)
                        for p in range(KC // 2):
                            nc.tensor.matmul(
                                ps[:, j, 0:Ct],
                                wa[:, 2 * p:2 * p + 2, 128 * j:128 * (j + 1)],
                                _tm(tgtT, ti, p),
                                start=False,
                                stop=(p == KC // 2 - 1 and last),
                                perf_mode=DR, skip_group_check=True)
                    else:
                        for p in range(KC // 2):
                            nc.tensor.matmul(
                                ps[:, j, 0:Ct],
                                wa[:, 2 * p:2 * p + 2, 128 * j:128 * (j + 1)],
                                _tm(tgtT, ti, p),
                                start=(p == 0 and first), stop=False,
                                perf_mode=DR, skip_group_check=True)
                        nc.tensor.matmul(
                            ps[:, j, 0:Ct], ident[:], xp[:, j, c0:c1],
                            start=False, stop=last, skip_group_check=True)
                if zb:
                    nc.scalar.activation(y[:, :, 0:Ct], ps[:, :, 0:Ct],
                                         AF.Copy, scale=unsc)
                else:
                    for j in range(KC):
                        nc.scalar.activation(
                            y[:, j, 0:Ct], ps[:, j, 0:Ct], AF.Copy,
                            scale=unsc, bias=par[:, pb + j:pb + j + 1])
                ys_all[l][ti] = y

            def ln1_stage(l, ti):
                pb = _P_LAYER + 40 * l
                t = tiles[ti]
                st = stats_stage(ys_all[l][ti], t[1] - t[0])
                ln_chain(ys_all[l][ti], st, t[1] - t[0], pb + 24, pb + 28,
                         xn_all[l][0], xf8_all[l], t, newton=False)

            def _f1_half(l, ti, half):
                pb = _P_LAYER + 40 * l
                f1w = f1s[l]
                xin = xf8_all[l] if FFN_FP8 else xn_all[l][0]
                c0, c1 = tiles[ti]
                Ct = c1 - c0
                f8dt_ = f8 if FFN_FP8 else bf16
                if half == 0:
                    hh = sb.tile([128, FKC, 512], f8dt_, tag="h", bufs=2)
                    hh_all[l][ti] = hh
                hh = hh_all[l][ti]
                for g in range(2 * half, 2 * half + 2):
                    ps = psp.tile([128, 4, 256], f32, tag="mm", bufs=3)
                    for j in range(KC):
                        mi = KC * g + j
                        first = (j % 2 == 0)
                        last = (j % 2 == 1)
                        for p in range(KC // 2):
                            nc.tensor.matmul(
                                ps[:, j, 0:Ct],
                                f1w[:, 2 * p:2 * p + 2,
                                    128 * mi:128 * (mi + 1)],
                                xin[:, 2 * p:2 * p + 2, c0:c1],
                                start=(p == 0 and first),
                                stop=(p == KC // 2 - 1 and last),
                                perf_mode=DR, skip_group_check=True)
                    if zb:
                        nc.scalar.activation(
                            hh[:, KC * g:KC * g + KC, 0:Ct], ps[:, :, 0:Ct],
                            AF.Gelu, scale=unsc)
                    else:
                        for j in range(KC):
                            mi = KC * g + j
                            nc.scalar.activation(
                                hh[:, mi, 0:Ct], ps[:, j, 0:Ct], AF.Gelu,
                                scale=unsc,
                                bias=par[:, pb + 4 + mi:pb + 4 + mi + 1])

            def f1a_stage(l, ti):
                _f1_half(l, ti, 0)

            def f1b_stage(l, ti):
                _f1_half(l, ti, 1)

            def f2_stage(l, ti):
                pb = _P_LAYER + 40 * l
                f2w = f2s[l]
                xn = xn_all[l][0]
                hh = hh_all[l][ti]
                c0, c1 = tiles[ti]
                Ct = c1 - c0
                y2 = sb.tile([128, KC, 512], bf16, tag="y", bufs=NT)
                ps = psp.tile([128, 4, 256], f32, tag="mm", bufs=3)
                for j in range(KC):
                    first = (j % 2 == 0)
                    last = (j % 2 == 1)
                    for p in range(FKC // 2):
                        nc.tensor.matmul(
                            ps[:, j, 0:Ct],
                            f2w[:, 2 * p:2 * p + 2, 128 * j:128 * (j + 1)],
                            hh[:, 2 * p:2 * p + 2, 0:Ct],
                            start=(p == 0 and first), stop=False,
                            perf_mode=DR, skip_group_check=True)
                    nc.tensor.matmul(
                        ps[:, j, 0:Ct], ident[:], xn[:, j, c0:c1],
                        start=False, stop=last, skip_group_check=True)
                if zb:
                    nc.scalar.activation(y2[:, :, 0:Ct], ps[:, :, 0:Ct],
                                         AF.Copy, scale=unsc)
                else:
                    for j in range(KC):
                        nc.scalar.activation(
                            y2[:, j, 0:Ct], ps[:, j, 0:Ct], AF.Copy,
                            scale=unsc,
                            bias=par[:, pb + 20 + j:pb + 20 + j + 1])
                y2s_all[l][ti] = y2

            def ln2_stage(l, ti):
                pb = _P_LAYER + 40 * l
                t = tiles[ti]
                st2 = stats_stage(y2s_all[l][ti], t[1] - t[0])
                ln_chain(y2s_all[l][ti], st2, t[1] - t[0], pb + 32, pb + 36,
                         xn_all[l][1],
                         xf8_all[l] if l == L - 1 else None, t,
                         newton=False)

            def out_stage(l, ti):
                xo = xf8_all[L - 1]
                c0, c1 = tiles[ti]
                Ct = c1 - c0
                ps = psp.tile([128, 4, 256], f32, tag="mm", bufs=3)
                for j in range(KC):
                    first = (j % 2 == 0)
                    last = (j % 2 == 1)
                    for p in range(KC // 2):
                        nc.tensor.matmul(
                            ps[:, j, 0:Ct],
                            opw[:, 2 * p:2 * p + 2, 128 * j:128 * (j + 1)],
                            xo[:, 2 * p:2 * p + 2, c0:c1],
                            start=(p == 0 and first),
                            stop=(p == KC // 2 - 1 and last),
                            perf_mode=DR, skip_group_check=True)
                ot = sb.tile([128, KC, 512], bf16, tag="o", bufs=2)
                if zb:
                    nc.scalar.activation(ot[:, :, 0:Ct], ps[:, :, 0:Ct],
                                         AF.Copy, scale=unsc)
                else:
                    for j in range(KC):
                        nc.scalar.activation(
                            ot[:, j, 0:Ct], ps[:, j, 0:Ct], AF.Copy,
                            scale=unsc,
                            bias=par[:, _P_OPB + j:_P_OPB + j + 1])
                oap = dram["out"].ap().rearrange("p (a b) -> p a b", a=KC)
                nc.sync.dma_start(oap[:, :, c0:c1], ot[:, :, 0:Ct])

            stage_fns = []
            for l in range(L):
                stage_fns += [
                    (attn_stage, l), (ln1_stage, l), (f1a_stage, l),
                    (f1b_stage, l), (f2_stage, l), (ln2_stage, l),
                ]
            stage_fns.append((out_stage, L - 1))
            NS = len(stage_fns)
            for wave in range(NS + NT - 1):
                for ls in range(NS - 1, -1, -1):
                    ti = wave - ls
                    if 0 <= ti < NT:
                        fn, l = stage_fns[ls]
                        fn(l, ti)

    nc.compile()
    return nc


_CACHE = {}


def _get_program(C, skips):
    fast = all(skips)
    key = (C, skips, fast)
    if key not in _CACHE:
        if fast:
            _CACHE[key] = _build_program_v2(C)
        else:
            _CACHE[key] = _build_program_general(C, skips)
    return _CACHE[key]


def _center(wT):
    """Center [K, M] weight over the output dim M so column sums of the
    produced activation vanish."""
    return wT - wT.mean(axis=1, keepdims=True)


def _prep_gen_weights(i, center, g_ipw, g_ipb, g_qkv_w, g_qkv_b, g_ao_w,
                      g_ao_b, g_ln1g, g_ln1b, g_ln2g, g_ln2b, g_f1w, g_f1b,
                      g_f2w, g_f2b, g_opw, g_opb, g_rw):
    wa, ba = [], []
    for l in range(L):
        _wq, _wk, wv = np.split(g_qkv_w[i, l], 3, axis=0)
        _bq, _bk, bv = np.split(g_qkv_b[i, l], 3)
        wa.append((g_ao_w[i, l] @ wv).T)                 # [K=H, M=H]
        ba.append(g_ao_b[i, l] + bv @ g_ao_w[i, l].T)
    rw = float(g_rw[i])
    ws = W8SCALE if FFN_FP8 else 1.0
    f8np = mybir.dt.np(f8 if FFN_FP8 else bf16)
    ipT = g_ipw[i].T.astype(np.float64)
    waT = [wa[l].astype(np.float64) for l in range(L)]
    f2T = [g_f2w[i, l].T.astype(np.float64) for l in range(L)]
    if center:
        ipT = _center(ipT)
        waT = [_center(w) for w in waT]
        f2T = [_center(w) for w in f2T]
    ipP = _sb_pack(W8SCALE * ipT, mybir.dt.np(f8))
    opP = _sb_pack(W8SCALE * (1.0 - rw) * g_opw[i].T, mybir.dt.np(f8))
    waP = np.stack([_sb_pack(W8SCALE * waT[l], mybir.dt.np(f8))
                    for l in range(L)])
    f1P = np.stack([_sb_pack(ws * g_f1w[i, l].T, f8np) for l in range(L)])
    f2P = np.stack([_sb_pack(ws * f2T[l], f8np) for l in range(L)])

    par = np.zeros((128, 128), np.float32)
    par[:, _P_IPB:_P_IPB + KC] = _pack_pcol(g_ipb[i])
    for l in range(L):
        pb = _P_LAYER + 40 * l
        bal = ba[l] + (g_ipb[i] if l == 0 else 0.0)   # layer-0 fuses ipb
        par[:, pb:pb + 4] = _pack_pcol(bal)
        par[:, pb + 4:pb + 20] = _pack_pcol(g_f1b[i, l])
        par[:, pb + 20:pb + 24] = _pack_pcol(g_f2b[i, l])
        par[:, pb + 24:pb + 28] = _pack_pcol(g_ln1g[i, l])
        par[:, pb + 28:pb + 32] = _pack_pcol(g_ln1b[i, l])
        par[:, pb + 32:pb + 36] = _pack_pcol(g_ln2g[i, l])
        par[:, pb + 36:pb + 40] = _pack_pcol(g_ln2b[i, l])
    par[:, _P_OPB:_P_OPB + KC] = _pack_pcol((1.0 - rw) * g_opb[i])

    zb = bool(np.all(g_ipb[i] == 0) and all(np.all(b == 0) for b in ba)
              and np.all(g_f1b[i] == 0) and np.all(g_f2b[i] == 0)
              and np.all(g_opb[i] == 0))
    ug = bool(np.all(g_ln1g[i] == 1) and np.all(g_ln2g[i] == 1))
    zbeta = bool(np.all(g_ln1b[i] == 0) and np.all(g_ln2b[i] == 0))
    return {"ip": ipP, "op": opP, "wa": waP, "f1": f1P, "f2": f2P,
            "par": par}, (zb, ug, zbeta), rw


def _gen_skips(i, g_ipb, g_qkv_w, g_qkv_b, g_ao_w, g_ao_b, g_ln1g, g_ln1b,
               g_ln2g, g_ln2b, g_f1b, g_f2b, g_opb, **_):
    ba = []
    for l in range(L):
        _bq, _bk, bv = np.split(g_qkv_b[i, l], 3)
        ba.append(g_ao_b[i, l] + bv @ g_ao_w[i, l].T)
    zb = bool(np.all(g_ipb[i] == 0) and all(np.all(b == 0) for b in ba)
              and np.all(g_f1b[i] == 0) and np.all(g_f2b[i] == 0)
              and np.all(g_opb[i] == 0))
    ug = bool(np.all(g_ln1g[i] == 1) and np.all(g_ln2g[i] == 1))
    zbeta = bool(np.all(g_ln1b[i] == 0) and np.all(g_ln2b[i] == 0))
    return (zb, ug, zbeta)


def _prepare(inputs):
    """Host-side prep. Returns (nc, in_maps, assemble)."""
    image = np.asarray(inputs["image_features"], np.float32)
    text = np.asarray(inputs["text_features"], np.float32)
    mt = np.asarray(inputs["missing_type"])

    idx1 = np.nonzero(mt == 1)[0]      # gen0 (img -> text) fills text
    idx2 = np.nonzero(mt == 2)[0]      # gen1 (text -> img) fills img
    idx3 = np.nonzero(mt == 3)[0]

    gw = {k: np.asarray(v) for k, v in inputs.items() if k.startswith("g_")}
    sk0 = _gen_skips(0, **{k: v for k, v in gw.items()
                           if k in ("g_ipb", "g_qkv_w", "g_qkv_b", "g_ao_w",
                                    "g_ao_b", "g_ln1g", "g_ln1b", "g_ln2g",
                                    "g_ln2b", "g_f1b", "g_f2b", "g_opb")})
    sk1 = _gen_skips(1, **{k: v for k, v in gw.items()
                           if k in ("g_ipb", "g_qkv_w", "g_qkv_b", "g_ao_w",
                                    "g_ao_b", "g_ln1g", "g_ln1b", "g_ln2g",
                                    "g_ln2b", "g_f1b", "g_f2b", "g_opb")})
    skips = tuple(a and b for a, b in zip(sk0, sk1))
    center = all(skips)
    w0, _, rw0 = _prep_gen_weights(0, center, **gw)
    w1, _, rw1 = _prep_gen_weights(1, center, **gw)

    # prior MLP on host (tiny)
    pe = np.asarray(inputs["prior_emb"], np.float64)
    t = pe @ np.asarray(inputs["prior_w1"], np.float64).T \
        + np.asarray(inputs["prior_b1"], np.float64)
    t = 0.5 * t * (1.0 + np.vectorize(math.erf)(t / math.sqrt(2.0)))
    prior = (t @ np.asarray(inputs["prior_w2"], np.float64).T
             + np.asarray(inputs["prior_b2"], np.float64)).astype(np.float32)
    p_img, p_text = prior[0, :H], prior[0, H:]

    imgT = np.ascontiguousarray(image.T)
    textT = np.ascontiguousarray(text.T)

    n_pc = -(-max(len(idx1), len(idx2), 1) // GCORES)   # per-core columns
    C = max(64, -(-n_pc // 16) * 16)                    # round up to 16

    tls = _tiles(C)

    def _pack_tm(M):
        """[H, C] -> tile-major [128, NT*KC*Tt] fp8."""
        a = M.astype(mybir.dt.np(f8)).reshape(KC, 128, C).transpose(1, 0, 2)
        return np.concatenate(
            [np.ascontiguousarray(a[:, :, t0:t1]).reshape(128, -1)
             for t0, t1 in tls], axis=1)

    def shard_cols(Tsrc, Ttgt, idx):
        pad = np.zeros(GCORES * C, np.int64)
        pad[:len(idx)] = idx
        pad = pad.reshape(GCORES, C)
        return [_pack_tm(Tsrc[:, pad[c]]) for c in range(GCORES)], \
            [_pack_tm(Ttgt[:, pad[c]]) for c in range(GCORES)]

    src0, tgt0 = shard_cols(imgT, textT, idx1)
    src1, tgt1 = shard_cols(textT, imgT, idx2)

    nc = _get_program(C, skips)

    ones = np.ones((128, 128), ml_dtypes.bfloat16)
    ident = (np.eye(128, dtype=np.float32) * W8SCALE).astype(ml_dtypes.bfloat16)
    id8 = np.zeros((128, 3 * 128), mybir.dt.np(f8))
    eye8 = (np.eye(128, dtype=np.float32) * W8SCALE).astype(mybir.dt.np(f8))
    id8[:, 0:128] = eye8      # slot 0: identity (even chunk of the DR pair)
    id8[:, 256:384] = eye8    # slot 2: identity (odd chunk of the DR pair)
    in_maps = []
    for c in range(N_CORES):
        g = 0 if c < GCORES else 1
        w = w0 if g == 0 else w1
        lc = c % GCORES
        im = {
            "src": (src0 if g == 0 else src1)[lc],
            "tgt": (tgt0 if g == 0 else tgt1)[lc],
            "ip": w["ip"], "op": w["op"], "wa": w["wa"], "f1": w["f1"],
            "f2": w["f2"], "ones": ones,
        }
        if center:
            im["id8"] = id8
        else:
            im["par"] = w["par"]
            im["ident"] = ident
        in_maps.append(im)

    def assemble(results):
        def gather_out(cores, idx, rw, full):
            cols = [np.asarray(results[c]["out"])
                    .astype(np.float32)
                    .reshape(128, KC, C).transpose(1, 0, 2).reshape(H, C)
                    for c in cores]
            allc = np.concatenate(cols, axis=1)[:, :len(idx)]
            return rw * full[idx] + allc.T

        enhanced_text = text.copy()
        if len(idx1):
            enhanced_text[idx1] = gather_out(range(GCORES), idx1, rw0, text)
        enhanced_img = image.copy()
        if len(idx2):
            enhanced_img[idx2] = gather_out(range(GCORES, N_CORES), idx2,
                                            rw1, image)
        if len(idx3):
            enhanced_img[idx3] = p_img
            enhanced_text[idx3] = p_text
        return enhanced_img, enhanced_text

    return nc, in_maps, assemble


def kernel(**inputs):
    nc, in_maps, assemble = _prepare(inputs)
    res = run_bass_kernel_spmd(nc, in_maps, list(range(N_CORES)))
    return assemble(res.results)


# revision 13
# speedup vs baseline: 1.0167x; 1.0167x over previous
"""Trainium2 Bass kernel for nn_ModalGenerator (MoE-routed cross-modal generator).

Strategy (v2):
  - seq_len==1 => attention collapses to v = tgt @ wv.T; fold wv/ao_w into one
    512x512 matrix per layer (host-side) and (1-rw) into the output projection.
  - MoE routing on host: gather missing_type==1 columns (gen0) and ==2 (gen1);
    missing_type==3 rows use the tiny host-computed prior MLP.
  - Generator-split sharding: cores 0-3 run generator 0 on 1/4 of its columns
    each, cores 4-7 run generator 1. Zero collectives (host gathers/scatters).
  - All projections run in fp8-e4m3 DoubleRow mode (K=256 per pass, 2x PE
    rate): weights scaled x64, the 1/64 unscale folded into downstream scales.
  - Zero-mean-by-construction LN (requires gamma==1, beta==0, zero biases,
    which setup_inputs always produces): LN output is exactly zero-mean, and
    every pre-LN activation is (centered GEMM) + (previous LN output), so
    centering ip/wa/f2 weights over their output dim on the host makes every
    pre-LN activation zero-mean. This removes the mean stats matmuls, the
    mean subtraction, and the identity-residual matmuls of v1 entirely:
      * residual adds ride the PSUM->SBUF copy (DVE scalar_tensor_tensor)
      * LN = sumsq stat (4 bf16 ones-matmuls) -> rstd via bf16 bit-hack with
        the 1/H folded into the magic constant (reads the high half-word of
        the f32 PSUM stat directly) -> single y*rstd apply.
  - Engine balance: PE GEMMs+stats; DVE psum copies/residuals, ysq, shift;
    Pool (gpsimd) magic op + LN apply muls (SBUF only - gpsimd cannot read
    PSUM); ACT gelu + fp8 copies of LN1 outputs + final-LN fp8.
  - Wavefront software pipelining across NT=5 column tiles (<=256 wide),
    10 stages per layer + out stage; later stages issue first within a wave.
  - PSUM: 3 "mm" buffers ([128,4,256] = 2 banks each) + 2 "st" single-bank
    stat buffers = 8 banks.
  - Fallback: if the instance has nonzero biases / non-unit gamma / nonzero
    beta, use the v1 general program (kept verbatim below).
"""

import math

import numpy as np
import ml_dtypes

import concourse.bacc as bacc
import concourse.mybir as mybir
import concourse.tile as tile
from concourse.bass_utils import run_bass_kernel_spmd

f32 = mybir.dt.float32
bf16 = mybir.dt.bfloat16
f8 = mybir.dt.float8e4
i16 = mybir.dt.int16
AF = mybir.ActivationFunctionType
ALU = mybir.AluOpType
DR = mybir.MatmulPerfMode.DoubleRow

H = 512
L = 3
N_CORES = 8
GCORES = 4               # cores per generator
KC = H // 128            # 4 k-chunks of the hidden dim
FH = 4 * H               # 2048 FFN hidden
FKC = FH // 128          # 16
LN_EPS = 1e-5
MAGIC16 = 0x5F37
MAGIC16_H = 0x5F37 + 576  # folds the 1/H (H=512=2^9) into the bit-hack
W8SCALE = 64.0           # fp8 weight pre-scale

FFN_FP8 = True

# param pack column layout: [128, 128] f32 (general path only)
_P_IPB = 0
_P_LAYER = 4             # + 40*l: ba 0..3 | f1b 4..19 | f2b 20..23
#                                 | ln1g 24..27 | ln1b 28..31 | ln2g 32..35 | ln2b 36..39
_P_OPB = 124


def _pack_pcol(vec):
    """[n*128] vector -> [128, n] chunk-column layout."""
    return np.ascontiguousarray(np.asarray(vec, np.float32).reshape(-1, 128).T)


def _sb_pack(wT, dt):
    """[K, M] (K mult of 128) -> [128, (K/128)*M] SBUF chunk-major layout."""
    K, M = wT.shape
    a = np.asarray(wT, np.float32).astype(dt)
    return np.ascontiguousarray(
        a.reshape(K // 128, 128, M).transpose(1, 0, 2).reshape(128, -1))


NT_TARGET = 6            # pipeline depth (equal column tiles per core)


def _tiles(C):
    nt = min(NT_TARGET, max(1, C // 64))
    base = C // nt // 16 * 16
    sizes = [base] * nt
    extra = C - base * nt
    i = 0
    while extra > 0:
        sizes[i] += min(16, extra)
        extra -= 16
        i = (i + 1) % nt
    # small first and last tiles (short serial pipeline fill and drain),
    # big middle tiles; cap 256 so 4 psum chunks pack into 2 banks
    sizes.sort(reverse=True)
    if nt >= 3:
        ends = [sizes.pop(), sizes.pop()]
        for i in range(2):
            for _ in range(3):
                j = sizes.index(min(sizes))
                if ends[i] - 16 >= 96 and sizes[j] + 16 <= 256:
                    ends[i] -= 16
                    sizes[j] += 16
        sizes.sort(reverse=True)
        sizes = [ends[0]] + sizes + [ends[1]]
    assert max(sizes) <= 256
    ts = []
    c0 = 0
    for s in sizes:
        if s > 0:
            ts.append((c0, c0 + s))
            c0 += s
    assert c0 == C
    return ts


def _build_program_v2(C):
    """Fast path: zero biases, unit gamma, zero beta (always true for the
    harness inputs). Weights ip/wa/f2 are centered host-side."""
    nc = bacc.Bacc("TRN2", target_bir_lowering=False, debug=False,
                   num_devices=N_CORES)

    dram = {
        "src": nc.dram_tensor("src", [128, KC * C], f8, kind="ExternalInput"),
        "tgt": nc.dram_tensor("tgt", [128, KC * C], f8, kind="ExternalInput"),
        "ip": nc.dram_tensor("ip", [128, KC * H], f8, kind="ExternalInput"),
        "op": nc.dram_tensor("op", [128, KC * H], f8, kind="ExternalInput"),
        "wa": nc.dram_tensor("wa", [L, 128, KC * H], f8, kind="ExternalInput"),
        "f1": nc.dram_tensor("f1", [L, 128, KC * FH], f8, kind="ExternalInput"),
        "f2": nc.dram_tensor("f2", [L, 128, FKC * H], f8, kind="ExternalInput"),
        "ones": nc.dram_tensor("ones", [128, 128], bf16, kind="ExternalInput"),
        "id8": nc.dram_tensor("id8", [128, 3 * 128], f8, kind="ExternalInput"),
        "out": nc.dram_tensor("out", [128, KC * C], bf16, kind="ExternalOutput"),
    }
    tiles = _tiles(C)
    NT = len(tiles)
    unsc = 1.0 / W8SCALE

    with tile.TileContext(nc) as tc:
        with (
            tc.tile_pool(name="sb", bufs=2) as sb,
            tc.tile_pool(name="ps", bufs=2, space="PSUM") as psp,
        ):
            ones = sb.tile([128, 128], bf16, tag="ones", bufs=1)
            nc.sync.dma_start(ones[:], dram["ones"].ap())
            id8 = sb.tile([128, 3, 128], f8, tag="id8", bufs=1)
            nc.sync.dma_start(id8[:], dram["id8"].ap().rearrange(
                "p (a b) -> p a b", a=3))
            ipw = sb.tile([128, KC, H], f8, tag="ip", bufs=1)
            nc.sync.dma_start(ipw[:], dram["ip"].ap())
            wa0 = sb.tile([128, KC, H], f8, tag="wa", bufs=2)
            nc.sync.dma_start(wa0[:], dram["wa"].ap()[0])
            srcT = sb.tile([128, KC * C], f8, tag="src", bufs=1)
            tgtT = sb.tile([128, KC * C], f8, tag="tgt", bufs=1)
            nc.sync.dma_start(srcT[:, 0:KC * tiles[0][1]],
                              dram["src"].ap()[:, 0:KC * tiles[0][1]])
            nc.sync.dma_start(tgtT[:, 0:KC * tiles[0][1]],
                              dram["tgt"].ap()[:, 0:KC * tiles[0][1]])
            # PE p-state warmup: keep the tensor engine continuously busy
            # from ~1us so real GEMMs start at the full 2.4 GHz clock, and
            # preload the ACT gelu/copy table during the DMA fill.
            warm = psp.tile([128, 512], f32, tag="st", bufs=2)
            for wi in range(30):
                nc.tensor.matmul(warm[:, 0:128], ones[:], ones[:],
                                 start=(wi == 0), stop=(wi == 29),
                                 skip_group_check=True)
            wact = sb.tile([128, 128], bf16, tag="wact", bufs=1)
            nc.scalar.activation(wact[:], ones[:], AF.Gelu)
            for ti in range(1, NT):
                c0, c1 = tiles[ti]
                nc.sync.dma_start(srcT[:, KC * c0:KC * c1],
                                  dram["src"].ap()[:, KC * c0:KC * c1])
                nc.sync.dma_start(tgtT[:, KC * c0:KC * c1],
                                  dram["tgt"].ap()[:, KC * c0:KC * c1])

            def _tm(flat, ti, p):
                c0, c1 = tiles[ti]
                Ct = c1 - c0
                sl = flat[:, KC * c0 + 2 * p * Ct:KC * c0 + (2 * p + 2) * Ct]
                return sl.rearrange("q (a b) -> q a b", a=2)

            was, f1s, f2s = [wa0], [], []
            for l in range(L):
                if l > 0:
                    wa = sb.tile([128, KC, H], f8, tag="wa", bufs=2)
                    nc.sync.dma_start(wa[:], dram["wa"].ap()[l])
                    was.append(wa)
                f1w = sb.tile([128, KC, FH], f8, tag="f1", bufs=2)
                nc.sync.dma_start(f1w[:], dram["f1"].ap()[l])
                f1s.append(f1w)
                f2w = sb.tile([128, FKC, H], f8, tag="f2", bufs=2)
                nc.sync.dma_start(f2w[:], dram["f2"].ap()[l])
                f2s.append(f2w)
            opw = sb.tile([128, KC, H], f8, tag="op", bufs=1)
            nc.sync.dma_start(opw[:], dram["op"].ap())

            # per-layer activation tensors (full C width, per-tile writes)
            xn_all = []          # ln2_out bf16 per layer (attn residual)
            xf8_all = []         # ln1_out fp8 (f1 input + f2 residual);
            #                      [L-1] reused for op input
            for l in range(L):
                xn_b = sb.tile([128, KC, C], bf16, tag="x", bufs=2)
                xn_all.append((None, xn_b))
                xf8_t = sb.tile([128, KC, C], f8, tag="xf8", bufs=2)
                xf8_all.append(xf8_t)

            ps_all = [[None] * NT for _ in range(L)]   # attn psum
            pf_all = [[None] * NT for _ in range(L)]   # f2 psum
            y1_all = [[None] * NT for _ in range(L)]
            y2_all = [[None] * NT for _ in range(L)]
            st1_all = [[None] * NT for _ in range(L)]
            st2_all = [[None] * NT for _ in range(L)]
            rs1_all = [[None] * NT for _ in range(L)]
            rs2_all = [[None] * NT for _ in range(L)]
            hh_all = [[None] * NT for _ in range(L)]

            def _attn_mm(l, ti):
                wa = was[l]
                c0, c1 = tiles[ti]
                Ct = c1 - c0
                ps = psp.tile([128, 4, 256], f32, tag="mm", bufs=3)
                ps_all[l][ti] = ps
                for j in range(KC):
                    first = (j % 2 == 0)
                    last = (j % 2 == 1)
                    if l == 0:
                        for p in range(KC // 2):
                            nc.tensor.matmul(
                                ps[:, j, 0:Ct],
                                ipw[:, 2 * p:2 * p + 2, 128 * j:128 * (j + 1)],
                                _tm(srcT, ti, p),
                                start=(p == 0 and first), stop=False,
                                perf_mode=DR, skip_group_check=True)
                        for p in range(KC // 2):
                            nc.tensor.matmul(
                                ps[:, j, 0:Ct],
                                wa[:, 2 * p:2 * p + 2, 128 * j:128 * (j + 1)],
                                _tm(tgtT, ti, p),
                                start=False,
                                stop=(p == KC // 2 - 1 and last),
                                perf_mode=DR, skip_group_check=True)
                    else:
                        for p in range(KC // 2):
                            nc.tensor.matmul(
                                ps[:, j, 0:Ct],
                                wa[:, 2 * p:2 * p + 2, 128 * j:128 * (j + 1)],
                                _tm(tgtT, ti, p),
                                start=(p == 0 and first),
                                stop=(p == KC // 2 - 1 and last),
                                perf_mode=DR, skip_group_check=True)

            def _y1(l, ti):
                c0, c1 = tiles[ti]
                Ct = c1 - c0
                ps = ps_all[l][ti]
                y1 = sb.tile([128, KC, 256], bf16, tag="y", bufs=2 * NT)
                y1_all[l][ti] = y1
                if l == 0:
                    nc.vector.tensor_scalar(y1[:, :, 0:Ct], ps[:, :, 0:Ct],
                                            unsc, None, ALU.mult)
                else:
                    xp = xn_all[l - 1][1]
                    nc.vector.scalar_tensor_tensor(
                        y1[:, :, 0:Ct], ps[:, :, 0:Ct], unsc,
                        xp[:, :, c0:c1], ALU.mult, ALU.add)

            def _sq(y, Ct, holder, l, ti):
                """ysq -> sumsq stat -> shifted high half-words (the stat is
                consumed here so its PSUM slot frees within the stage)."""
                ysq = sb.tile([128, KC, 256], bf16, tag="ysq", bufs=3)
                nc.vector.tensor_mul(ysq[:, :, 0:Ct], y[:, :, 0:Ct],
                                     y[:, :, 0:Ct])
                st = psp.tile([128, 512], f32, tag="st", bufs=2)
                for k in range(KC):
                    nc.tensor.matmul(st[:, 0:Ct], ones[:], ysq[:, k, 0:Ct],
                                     start=(k == 0), stop=(k == KC - 1),
                                     skip_group_check=True)
                sh = sb.tile([128, 256], i16, tag="sh", bufs=4)
                st16 = st.bitcast(i16)          # [128, 1024]
                nc.vector.tensor_scalar(sh[:, 0:Ct], st16[:, 1:2 * Ct:2],
                                        1, None, ALU.logical_shift_right)
                holder[l][ti] = sh

            def _rstd(sh, Ct):
                """rstd = magic - (bits >> 1), 1/H folded into the magic."""
                rstd = sb.tile([128, 256], bf16, tag="rstd", bufs=4)
                nc.vector.tensor_scalar(rstd[:, 0:Ct].bitcast(i16),
                                        sh[:, 0:Ct], -1, MAGIC16_H,
                                        ALU.mult, ALU.add)
                return rstd

            def s1_stage(l, ti):
                _attn_mm(l, ti)

            def s1b_stage(l, ti):
                _y1(l, ti)

            def s2_stage(l, ti):
                c0, c1 = tiles[ti]
                _sq(y1_all[l][ti], c1 - c0, st1_all, l, ti)

            def s3_stage(l, ti):
                c0, c1 = tiles[ti]
                Ct = c1 - c0
                y1 = y1_all[l][ti]
                rstd = _rstd(st1_all[l][ti], Ct)
                rs1_all[l][ti] = rstd
                r4 = rstd[:, 0:Ct].unsqueeze(1).broadcast_to((128, KC, Ct))
                nc.gpsimd.tensor_mul(xf8_all[l][:, :, c0:c1],
                                     y1[:, :, 0:Ct], r4)

            def f1a_stage(l, ti):
                _f1_half(l, ti, 0)

            def _f1_half(l, ti, half):
                f1w = f1s[l]
                xin = xf8_all[l]
                c0, c1 = tiles[ti]
                Ct = c1 - c0
                if half == 0:
                    hh = sb.tile([128, FKC, 512], f8, tag="h", bufs=2)
                    hh_all[l][ti] = hh
                hh = hh_all[l][ti]
                for g in range(2 * half, 2 * half + 2):
                    ps = psp.tile([128, 4, 256], f32, tag="mm", bufs=3)
                    for j in range(KC):
                        mi = KC * g + j
                        first = (j % 2 == 0)
                        last = (j % 2 == 1)
                        for p in range(KC // 2):
                            nc.tensor.matmul(
                                ps[:, j, 0:Ct],
                                f1w[:, 2 * p:2 * p + 2,
                                    128 * mi:128 * (mi + 1)],
                                xin[:, 2 * p:2 * p + 2, c0:c1],
                                start=(p == 0 and first),
                                stop=(p == KC // 2 - 1 and last),
                                perf_mode=DR, skip_group_check=True)
                    nc.scalar.activation(
                        hh[:, KC * g:KC * g + KC, 0:Ct], ps[:, :, 0:Ct],
                        AF.Gelu, scale=unsc)

            def f1b_stage(l, ti):
                _f1_half(l, ti, 1)

            def f2_stage(l, ti):
                f2w = f2s[l]
                hh = hh_all[l][ti]
                xf8 = xf8_all[l]
                c0, c1 = tiles[ti]
                Ct = c1 - c0
                ps = psp.tile([128, 4, 256], f32, tag="mm", bufs=3)
                pf_all[l][ti] = ps
                for j in range(KC):
                    first = (j % 2 == 0)
                    last = (j % 2 == 1)
                    for p in range(FKC // 2):
                        nc.tensor.matmul(
                            ps[:, j, 0:Ct],
                            f2w[:, 2 * p:2 * p + 2, 128 * j:128 * (j + 1)],
                            hh[:, 2 * p:2 * p + 2, 0:Ct],
                            start=(p == 0 and first), stop=False,
                            perf_mode=DR, skip_group_check=True)
                    # residual: 64*I (fp8 DR) on the LN1 fp8 output
                    pr = j // 2
                    nc.tensor.matmul(
                        ps[:, j, 0:Ct],
                        id8[:, (0 if j % 2 == 0 else 1):(2 if j % 2 == 0 else 3), :],
                        xf8[:, 2 * pr:2 * pr + 2, c0:c1],
                        start=False, stop=last,
                        perf_mode=DR, skip_group_check=True)

            def y2_stage(l, ti):
                c0, c1 = tiles[ti]
                Ct = c1 - c0
                ps = pf_all[l][ti]
                y2 = sb.tile([128, KC, 256], bf16, tag="y", bufs=2 * NT)
                y2_all[l][ti] = y2
                nc.vector.tensor_scalar(y2[:, :, 0:Ct], ps[:, :, 0:Ct],
                                        unsc, None, ALU.mult)

            def sq2_stage(l, ti):
                c0, c1 = tiles[ti]
                _sq(y2_all[l][ti], c1 - c0, st2_all, l, ti)

            def ap2_stage(l, ti):
                c0, c1 = tiles[ti]
                Ct = c1 - c0
                y2 = y2_all[l][ti]
                rstd = _rstd(st2_all[l][ti], Ct)
                rs2_all[l][ti] = rstd
                r4 = rstd[:, 0:Ct].unsqueeze(1).broadcast_to((128, KC, Ct))
                if l < L - 1:
                    xn2 = xn_all[l][1]
                    nc.gpsimd.tensor_mul(xn2[:, :, c0:c1],
                                         y2[:, :, 0:Ct], r4)
                else:
                    # final LN feeds only the output projection: fp8
                    nc.vector.tensor_mul(xf8_all[L - 1][:, :, c0:c1],
                                         y2[:, :, 0:Ct], r4)

            def out_stage(l, ti):
                xo = xf8_all[L - 1]
                c0, c1 = tiles[ti]
                Ct = c1 - c0
                ps = psp.tile([128, 4, 256], f32, tag="mm", bufs=3)
                for j in range(KC):
                    first = (j % 2 == 0)
                    last = (j % 2 == 1)
                    for p in range(KC // 2):
                        nc.tensor.matmul(
                            ps[:, j, 0:Ct],
                            opw[:, 2 * p:2 * p + 2, 128 * j:128 * (j + 1)],
                            xo[:, 2 * p:2 * p + 2, c0:c1],
                            start=(p == 0 and first),
                            stop=(p == KC // 2 - 1 and last),
                            perf_mode=DR, skip_group_check=True)
                ot = sb.tile([128, KC, 512], bf16, tag="o", bufs=2)
                nc.scalar.activation(ot[:, :, 0:Ct], ps[:, :, 0:Ct],
                                     AF.Copy, scale=unsc)
                oap = dram["out"].ap().rearrange("p (a b) -> p a b", a=KC)
                nc.sync.dma_start(oap[:, :, c0:c1], ot[:, :, 0:Ct])

            # wavefront issue: stage ls of tile t at wave ls + t
            stage_fns = []
            for l in range(L):
                stage_fns += [
                    (s1_stage, l), (s1b_stage, l), (s2_stage, l),
                    (s3_stage, l), (f1a_stage, l), (f1b_stage, l),
                    (f2_stage, l), (y2_stage, l), (sq2_stage, l),
                    (ap2_stage, l),
                ]
            stage_fns.append((out_stage, L - 1))
            NS = len(stage_fns)
            # tiles spaced SPREAD waves apart: the active stage window spans
            # SPREAD*NT stages, covering a whole layer cycle with NT=5
            SPREAD = 2
            for wave in range(NS + SPREAD * (NT - 1) + 1):
                for t in range(0, NT):
                    s = wave - SPREAD * t
                    if 0 <= s < NS:
                        fn, l = stage_fns[s]
                        fn(l, t)

    nc.compile()
    return nc


def _build_program_general(C, skips):
    """v1 general program (biases / gamma / beta supported). Verbatim from
    the baseline kernel."""
    zb, ug, zbeta = skips
    f8dt = f8 if FFN_FP8 else bf16
    nc = bacc.Bacc("TRN2", target_bir_lowering=False, debug=False,
                   num_devices=N_CORES)

    dram = {
        "src": nc.dram_tensor("src", [128, KC * C], f8, kind="ExternalInput"),
        "tgt": nc.dram_tensor("tgt", [128, KC * C], f8, kind="ExternalInput"),
        "ip": nc.dram_tensor("ip", [128, KC * H], f8, kind="ExternalInput"),
        "op": nc.dram_tensor("op", [128, KC * H], f8, kind="ExternalInput"),
        "wa": nc.dram_tensor("wa", [L, 128, KC * H], f8, kind="ExternalInput"),
        "f1": nc.dram_tensor("f1", [L, 128, KC * FH], f8dt, kind="ExternalInput"),
        "f2": nc.dram_tensor("f2", [L, 128, FKC * H], f8dt, kind="ExternalInput"),
        "par": nc.dram_tensor("par", [128, 128], f32, kind="ExternalInput"),
        "ones": nc.dram_tensor("ones", [128, 128], bf16, kind="ExternalInput"),
        "ident": nc.dram_tensor("ident", [128, 128], bf16, kind="ExternalInput"),
        "out": nc.dram_tensor("out", [128, KC * C], bf16, kind="ExternalOutput"),
    }
    tiles = _tiles(C)
    NT = len(tiles)
    unsc = 1.0 / W8SCALE if FFN_FP8 else 1.0

    with tile.TileContext(nc) as tc:
        with (
            tc.tile_pool(name="sb", bufs=2) as sb,
            tc.tile_pool(name="ps", bufs=2, space="PSUM") as psp,
        ):
            ipw = sb.tile([128, KC, H], f8, tag="ip", bufs=1)
            nc.sync.dma_start(ipw[:], dram["ip"].ap())
            wa0 = sb.tile([128, KC, H], f8, tag="wa", bufs=2)
            nc.sync.dma_start(wa0[:], dram["wa"].ap()[0])
            srcT = sb.tile([128, KC * C], f8, tag="src", bufs=1)
            tgtT = sb.tile([128, KC * C], f8, tag="tgt", bufs=1)
            nc.sync.dma_start(srcT[:, 0:KC * tiles[0][1]],
                              dram["src"].ap()[:, 0:KC * tiles[0][1]])
            nc.sync.dma_start(tgtT[:, 0:KC * tiles[0][1]],
                              dram["tgt"].ap()[:, 0:KC * tiles[0][1]])
            ones = sb.tile([128, 128], bf16, tag="ones", bufs=1)
            nc.sync.dma_start(ones[:], dram["ones"].ap())
            ident = sb.tile([128, 128], bf16, tag="ident", bufs=1)
            nc.sync.dma_start(ident[:], dram["ident"].ap())
            for ti in range(1, NT):
                c0, c1 = tiles[ti]
                nc.sync.dma_start(srcT[:, KC * c0:KC * c1],
                                  dram["src"].ap()[:, KC * c0:KC * c1])
                nc.sync.dma_start(tgtT[:, KC * c0:KC * c1],
                                  dram["tgt"].ap()[:, KC * c0:KC * c1])

            def _tm(flat, ti, p):
                c0, c1 = tiles[ti]
                Ct = c1 - c0
                sl = flat[:, KC * c0 + 2 * p * Ct:KC * c0 + (2 * p + 2) * Ct]
                return sl.rearrange("q (a b) -> q a b", a=2)
            par = sb.tile([128, 128], f32, tag="par", bufs=1)
            nc.sync.dma_start(par[:], dram["par"].ap())

            def stats_stage(y, Ct):
                ysq = sb.tile([128, KC, 512], bf16, tag="ysq", bufs=2)
                nc.vector.tensor_mul(ysq[:, :, 0:Ct], y[:, :, 0:Ct],
                                     y[:, :, 0:Ct])
                st = psp.tile([128, 2, 256], f32, tag="st", bufs=2)
                for k in range(KC):
                    nc.tensor.matmul(st[:, 0, 0:Ct], ones[:], y[:, k, 0:Ct],
                                     start=(k == 0), stop=False,
                                     skip_group_check=True)
                for k in range(KC):
                    nc.tensor.matmul(st[:, 1, 0:Ct], ones[:], ysq[:, k, 0:Ct],
                                     start=False, stop=(k == KC - 1),
                                     skip_group_check=True)
                return st

            def ln_chain(y, st, Ct, gcol, bcol, xn, xf8, t, newton=True):
                c0, c1 = t
                m = sb.tile([128, 512], bf16, tag="m", bufs=2)
                nc.vector.tensor_scalar(m[:, 0:Ct], st[:, 0, 0:Ct], 1.0 / H,
                                        None, ALU.mult)
                msq = sb.tile([128, 512], bf16, tag="msq", bufs=2)
                nc.vector.scalar_tensor_tensor(msq[:, 0:Ct], st[:, 0, 0:Ct],
                                               1.0 / H, m[:, 0:Ct],
                                               ALU.mult, ALU.mult)
                z = sb.tile([128, 512], bf16, tag="z", bufs=2)
                nc.vector.scalar_tensor_tensor(z[:, 0:Ct], st[:, 1, 0:Ct],
                                               1.0 / H, msq[:, 0:Ct],
                                               ALU.mult, ALU.subtract)
                r = sb.tile([128, 512], bf16, tag="rx", bufs=2)
                nc.vector.tensor_scalar(r[:, 0:Ct].bitcast(i16),
                                        z[:, 0:Ct].bitcast(i16), 1, None,
                                        ALU.logical_shift_right)
                rstd = sb.tile([128, 512], bf16, tag="rstd", bufs=4)
                nc.vector.tensor_scalar(rstd[:, 0:Ct].bitcast(i16),
                                        r[:, 0:Ct].bitcast(i16), -1,
                                        MAGIC16, ALU.mult, ALU.add)
                if newton:
                    u = sb.tile([128, 512], bf16, tag="u", bufs=2)
                    nc.vector.tensor_mul(u[:, 0:Ct], rstd[:, 0:Ct],
                                         rstd[:, 0:Ct])
                    w = sb.tile([128, 512], bf16, tag="w", bufs=2)
                    nc.vector.scalar_tensor_tensor(w[:, 0:Ct], u[:, 0:Ct],
                                                   -0.5, z[:, 0:Ct],
                                                   ALU.mult, ALU.mult)
                    rstd2 = sb.tile([128, 512], bf16, tag="rstd", bufs=4)
                    nc.vector.scalar_tensor_tensor(rstd2[:, 0:Ct], w[:, 0:Ct],
                                                   1.5, rstd[:, 0:Ct],
                                                   ALU.add, ALU.mult)
                else:
                    rstd2 = rstd
                m4 = m[:, 0:Ct].unsqueeze(1).broadcast_to((128, KC, Ct))
                r4 = rstd2[:, 0:Ct].unsqueeze(1).broadcast_to((128, KC, Ct))
                if ug and zbeta:
                    u1 = sb.tile([128, KC, 512], bf16, tag="u1", bufs=2)
                    nc.vector.tensor_sub(u1[:, :, 0:Ct], y[:, :, 0:Ct], m4)
                    if xf8 is not None:
                        nc.gpsimd.tensor_mul(xf8[:, :, c0:c1],
                                             u1[:, :, 0:Ct], r4)
                        nc.vector.tensor_mul(xn[:, :, c0:c1],
                                             u1[:, :, 0:Ct], r4)
                    else:
                        nc.vector.tensor_mul(xn[:, :, c0:c1],
                                             u1[:, :, 0:Ct], r4)
                else:
                    for mm in range(KC):
                        u1 = sb.tile([128, 512], bf16, tag="u1c", bufs=1)
                        nc.vector.tensor_sub(u1[:, 0:Ct], y[:, mm, 0:Ct],
                                             m[:, 0:Ct])
                        u2 = sb.tile([128, 512], bf16, tag="u2c", bufs=1)
                        nc.vector.scalar_tensor_tensor(
                            u2[:, 0:Ct], u1[:, 0:Ct],
                            par[:, gcol + mm:gcol + mm + 1], rstd2[:, 0:Ct],
                            ALU.mult, ALU.mult)
                        nc.vector.tensor_scalar(
                            xn[:, mm, c0:c1], u2[:, 0:Ct],
                            par[:, bcol + mm:bcol + mm + 1], None, ALU.add)
                    if xf8 is not None:
                        nc.scalar.activation(xf8[:, :, c0:c1], xn[:, :, c0:c1],
                                             AF.Copy)

            x = None
            xn_all, xf8_all = [], []
            for l in range(L):
                xn_a = sb.tile([128, KC, C], bf16, tag="x", bufs=3)
                xn_b = sb.tile([128, KC, C], bf16, tag="x", bufs=3)
                xn_all.append((xn_a, xn_b))
                if FFN_FP8:
                    xf8_t = sb.tile([128, KC, C], f8, tag="xf8", bufs=2)
                    xf8_all.append(xf8_t)
                else:
                    xf8_all.append(None)

            was, f1s, f2s = [wa0], [], []
            for l in range(L):
                if l > 0:
                    wa = sb.tile([128, KC, H], f8, tag="wa", bufs=2)
                    nc.sync.dma_start(wa[:], dram["wa"].ap()[l])
                    was.append(wa)
                f1w = sb.tile([128, KC, FH], f8dt, tag="f1", bufs=2)
                nc.sync.dma_start(f1w[:], dram["f1"].ap()[l])
                f1s.append(f1w)
                f2w = sb.tile([128, FKC, H], f8dt, tag="f2", bufs=2)
                nc.sync.dma_start(f2w[:], dram["f2"].ap()[l])
                f2s.append(f2w)
            opw = sb.tile([128, KC, H], f8, tag="op", bufs=1)
            nc.sync.dma_start(opw[:], dram["op"].ap())

            ys_all = [[None] * NT for _ in range(L)]
            y2s_all = [[None] * NT for _ in range(L)]
            hh_all = [[None] * NT for _ in range(L)]

            def attn_stage(l, ti):
                pb = _P_LAYER + 40 * l
                wa = was[l]
                xp = xn_all[l - 1][1] if l > 0 else None
                c0, c1 = tiles[ti]
                Ct = c1 - c0
                y = sb.tile([128, KC, 512], bf16, tag="y", bufs=NT)
                ps = psp.tile([128, 4, 256], f32, tag="mm", bufs=3)
                for j in range(KC):
                    first = (j % 2 == 0)
                    last = (j % 2 == 1)
                    if l == 0:
                        for p in range(KC // 2):
                            nc.tensor.matmul(
                                ps[:, j, 0:Ct],
                                ipw[:, 2 * p:2 * p + 2, 128 * j:128 * (j + 1)],
                                _tm(srcT, ti, p),
                                start=(p == 0 and first), stop=False,
                                perf_mode=DR, skip_group_check=True)
                        for p in range(KC // 2):
                            nc.tensor.matmul(
                                ps[:, j, 0:Ct],
                                wa[:, 2 * p:2 * p + 2, 128 * j:128 * (j + 1)],
                                _tm(tgtT, ti, p),
                                start=False,
                                stop=(p == KC // 2 - 1 and last),
                                perf_mode=DR, skip_group_check=True)
                    else:
                        for p in range(KC // 2):
                            nc.tensor.matmul(
                                ps[:, j, 0:Ct],
                                wa[:, 2 * p:2 * p + 2, 128 * j:128 * (j + 1)],
                                _tm(tgtT, ti, p),
                                start=(p == 0 and first), stop=False,
                                perf_mode=DR, skip_group_check=True)
                        nc.tensor.matmul(
                            ps[:, j, 0:Ct], ident[:], xp[:, j, c0:c1],
                            start=False, stop=last, skip_group_check=True)
                if zb:
                    nc.scalar.activation(y[:, :, 0:Ct], ps[:, :, 0:Ct],
                                         AF.Copy, scale=unsc)
                else:
                    for j in range(KC):
                        nc.scalar.activation(
                            y[:, j, 0:Ct], ps[:, j, 0:Ct], AF.Copy,
                            scale=unsc, bias=par[:, pb + j:pb + j + 1])
                ys_all[l][ti] = y

            def ln1_stage(l, ti):
                pb = _P_LAYER + 40 * l
                t = tiles[ti]
                st = stats_stage(ys_all[l][ti], t[1] - t[0])
                ln_chain(ys_all[l][ti], st, t[1] - t[0], pb + 24, pb + 28,
                         xn_all[l][0], xf8_all[l], t, newton=False)

            def _f1_half(l, ti, half):
                pb = _P_LAYER + 40 * l
                f1w = f1s[l]
                xin = xf8_all[l] if FFN_FP8 else xn_all[l][0]
                c0, c1 = tiles[ti]
                Ct = c1 - c0
                f8dt_ = f8 if FFN_FP8 else bf16
                if half == 0:
                    hh = sb.tile([128, FKC, 512], f8dt_, tag="h", bufs=2)
                    hh_all[l][ti] = hh
                hh = hh_all[l][ti]
                for g in range(2 * half, 2 * half + 2):
                    ps = psp.tile([128, 4, 256], f32, tag="mm", bufs=3)
                    for j in range(KC):
                        mi = KC * g + j
                        first = (j % 2 == 0)
                        last = (j % 2 == 1)
                        for p in range(KC // 2):
                            nc.tensor.matmul(
                                ps[:, j, 0:Ct],
                                f1w[:, 2 * p:2 * p + 2,
                                    128 * mi:128 * (mi + 1)],
                                xin[:, 2 * p:2 * p + 2, c0:c1],
                                start=(p == 0 and first),
                                stop=(p == KC // 2 - 1 and last),
                                perf_mode=DR, skip_group_check=True)
                    if zb:
                        nc.scalar.activation(
                            hh[:, KC * g:KC * g + KC, 0:Ct], ps[:, :, 0:Ct],
                            AF.Gelu, scale=unsc)
                    else:
                        for j in range(KC):
                            mi = KC * g + j
                            nc.scalar.activation(
                                hh[:, mi, 0:Ct], ps[:, j, 0:Ct], AF.Gelu,
                                scale=unsc,
                                bias=par[:, pb + 4 + mi:pb + 4 + mi + 1])

            def f1a_stage(l, ti):
                _f1_half(l, ti, 0)

            def f1b_stage(l, ti):
                _f1_half(l, ti, 1)

            def f2_stage(l, ti):
                pb = _P_LAYER + 40 * l
                f2w = f2s[l]
                xn = xn_all[l][0]
                hh = hh_all[l][ti]
                c0, c1 = tiles[ti]
                Ct = c1 - c0
                y2 = sb.tile([128, KC, 512], bf16, tag="y", bufs=NT)
                ps = psp.tile([128, 4, 256], f32, tag="mm", bufs=3)
                for j in range(KC):
                    first = (j % 2 == 0)
                    last = (j % 2 == 1)
                    for p in range(FKC // 2):
                        nc.tensor.matmul(
                            ps[:, j, 0:Ct],
                            f2w[:, 2 * p:2 * p + 2, 128 * j:128 * (j + 1)],
                            hh[:, 2 * p:2 * p + 2, 0:Ct],
                            start=(p == 0 and first), stop=False,
                            perf_mode=DR, skip_group_check=True)
                    nc.tensor.matmul(
                        ps[:, j, 0:Ct], ident[:], xn[:, j, c0:c1],
                        start=False, stop=last, skip_group_check=True)
                if zb:
                    nc.scalar.activation(y2[:, :, 0:Ct], ps[:, :, 0:Ct],
                                         AF.Copy, scale=unsc)
                else:
                    for j in range(KC):
                        nc.scalar.activation(
                            y2[:, j, 0:Ct], ps[:, j, 0:Ct], AF.Copy,
                            scale=unsc,
                            bias=par[:, pb + 20 + j:pb + 20 + j + 1])
                y2s_all[l][ti] = y2

            def ln2_stage(l, ti):
                pb = _P_LAYER + 40 * l
                t = tiles[ti]
                st2 = stats_stage(y2s_all[l][ti], t[1] - t[0])
                ln_chain(y2s_all[l][ti], st2, t[1] - t[0], pb + 32, pb + 36,
                         xn_all[l][1],
                         xf8_all[l] if l == L - 1 else None, t,
                         newton=False)

            def out_stage(l, ti):
                xo = xf8_all[L - 1]
                c0, c1 = tiles[ti]
                Ct = c1 - c0
                ps = psp.tile([128, 4, 256], f32, tag="mm", bufs=3)
                for j in range(KC):
                    first = (j % 2 == 0)
                    last = (j % 2 == 1)
                    for p in range(KC // 2):
                        nc.tensor.matmul(
                            ps[:, j, 0:Ct],
                            opw[:, 2 * p:2 * p + 2, 128 * j:128 * (j + 1)],
                            xo[:, 2 * p:2 * p + 2, c0:c1],
                            start=(p == 0 and first),
                            stop=(p == KC // 2 - 1 and last),
                            perf_mode=DR, skip_group_check=True)
                ot = sb.tile([128, KC, 512], bf16, tag="o", bufs=2)
                if zb:
                    nc.scalar.activation(ot[:, :, 0:Ct], ps[:, :, 0:Ct],
                                         AF.Copy, scale=unsc)
                else:
                    for j in range(KC):
                        nc.scalar.activation(
                            ot[:, j, 0:Ct], ps[:, j, 0:Ct], AF.Copy,
                            scale=unsc,
                            bias=par[:, _P_OPB + j:_P_OPB + j + 1])
                oap = dram["out"].ap().rearrange("p (a b) -> p a b", a=KC)
                nc.sync.dma_start(oap[:, :, c0:c1], ot[:, :, 0:Ct])

            stage_fns = []
            for l in range(L):
                stage_fns += [
                    (attn_stage, l), (ln1_stage, l), (f1a_stage, l),
                    (f1b_stage, l), (f2_stage, l), (ln2_stage, l),
                ]
            stage_fns.append((out_stage, L - 1))
            NS = len(stage_fns)
            for wave in range(NS + NT - 1):
                for ls in range(NS - 1, -1, -1):
                    ti = wave - ls
                    if 0 <= ti < NT:
                        fn, l = stage_fns[ls]
                        fn(l, ti)

    nc.compile()
    return nc


_CACHE = {}


def _get_program(C, skips):
    fast = all(skips)
    key = (C, skips, fast)
    if key not in _CACHE:
        if fast:
            _CACHE[key] = _build_program_v2(C)
        else:
            _CACHE[key] = _build_program_general(C, skips)
    return _CACHE[key]


def _center(wT):
    """Center [K, M] weight over the output dim M so column sums of the
    produced activation vanish."""
    return wT - wT.mean(axis=1, keepdims=True)


def _prep_gen_weights(i, center, g_ipw, g_ipb, g_qkv_w, g_qkv_b, g_ao_w,
                      g_ao_b, g_ln1g, g_ln1b, g_ln2g, g_ln2b, g_f1w, g_f1b,
                      g_f2w, g_f2b, g_opw, g_opb, g_rw):
    wa, ba = [], []
    for l in range(L):
        _wq, _wk, wv = np.split(g_qkv_w[i, l], 3, axis=0)
        _bq, _bk, bv = np.split(g_qkv_b[i, l], 3)
        wa.append((g_ao_w[i, l] @ wv).T)                 # [K=H, M=H]
        ba.append(g_ao_b[i, l] + bv @ g_ao_w[i, l].T)
    rw = float(g_rw[i])
    ws = W8SCALE if FFN_FP8 else 1.0
    f8np = mybir.dt.np(f8 if FFN_FP8 else bf16)
    ipT = g_ipw[i].T.astype(np.float64)
    waT = [wa[l].astype(np.float64) for l in range(L)]
    f2T = [g_f2w[i, l].T.astype(np.float64) for l in range(L)]
    if center:
        ipT = _center(ipT)
        waT = [_center(w) for w in waT]
        f2T = [_center(w) for w in f2T]
    ipP = _sb_pack(W8SCALE * ipT, mybir.dt.np(f8))
    opP = _sb_pack(W8SCALE * (1.0 - rw) * g_opw[i].T, mybir.dt.np(f8))
    waP = np.stack([_sb_pack(W8SCALE * waT[l], mybir.dt.np(f8))
                    for l in range(L)])
    f1P = np.stack([_sb_pack(ws * g_f1w[i, l].T, f8np) for l in range(L)])
    f2P = np.stack([_sb_pack(ws * f2T[l], f8np) for l in range(L)])

    par = np.zeros((128, 128), np.float32)
    par[:, _P_IPB:_P_IPB + KC] = _pack_pcol(g_ipb[i])
    for l in range(L):
        pb = _P_LAYER + 40 * l
        bal = ba[l] + (g_ipb[i] if l == 0 else 0.0)   # layer-0 fuses ipb
        par[:, pb:pb + 4] = _pack_pcol(bal)
        par[:, pb + 4:pb + 20] = _pack_pcol(g_f1b[i, l])
        par[:, pb + 20:pb + 24] = _pack_pcol(g_f2b[i, l])
        par[:, pb + 24:pb + 28] = _pack_pcol(g_ln1g[i, l])
        par[:, pb + 28:pb + 32] = _pack_pcol(g_ln1b[i, l])
        par[:, pb + 32:pb + 36] = _pack_pcol(g_ln2g[i, l])
        par[:, pb + 36:pb + 40] = _pack_pcol(g_ln2b[i, l])
    par[:, _P_OPB:_P_OPB + KC] = _pack_pcol((1.0 - rw) * g_opb[i])

    zb = bool(np.all(g_ipb[i] == 0) and all(np.all(b == 0) for b in ba)
              and np.all(g_f1b[i] == 0) and np.all(g_f2b[i] == 0)
              and np.all(g_opb[i] == 0))
    ug = bool(np.all(g_ln1g[i] == 1) and np.all(g_ln2g[i] == 1))
    zbeta = bool(np.all(g_ln1b[i] == 0) and np.all(g_ln2b[i] == 0))
    return {"ip": ipP, "op": opP, "wa": waP, "f1": f1P, "f2": f2P,
            "par": par}, (zb, ug, zbeta), rw


def _gen_skips(i, g_ipb, g_qkv_w, g_qkv_b, g_ao_w, g_ao_b, g_ln1g, g_ln1b,
               g_ln2g, g_ln2b, g_f1b, g_f2b, g_opb, **_):
    ba = []
    for l in range(L):
        _bq, _bk, bv = np.split(g_qkv_b[i, l], 3)
        ba.append(g_ao_b[i, l] + bv @ g_ao_w[i, l].T)
    zb = bool(np.all(g_ipb[i] == 0) and all(np.all(b == 0) for b in ba)
              and np.all(g_f1b[i] == 0) and np.all(g_f2b[i] == 0)
              and np.all(g_opb[i] == 0))
    ug = bool(np.all(g_ln1g[i] == 1) and np.all(g_ln2g[i] == 1))
    zbeta = bool(np.all(g_ln1b[i] == 0) and np.all(g_ln2b[i] == 0))
    return (zb, ug, zbeta)


def _prepare(inputs):
    """Host-side prep. Returns (nc, in_maps, assemble)."""
    image = np.asarray(inputs["image_features"], np.float32)
    text = np.asarray(inputs["text_features"], np.float32)
    mt = np.asarray(inputs["missing_type"])

    idx1 = np.nonzero(mt == 1)[0]      # gen0 (img -> text) fills text
    idx2 = np.nonzero(mt == 2)[0]      # gen1 (text -> img) fills img
    idx3 = np.nonzero(mt == 3)[0]

    gw = {k: np.asarray(v) for k, v in inputs.items() if k.startswith("g_")}
    sk0 = _gen_skips(0, **{k: v for k, v in gw.items()
                           if k in ("g_ipb", "g_qkv_w", "g_qkv_b", "g_ao_w",
                                    "g_ao_b", "g_ln1g", "g_ln1b", "g_ln2g",
                                    "g_ln2b", "g_f1b", "g_f2b", "g_opb")})
    sk1 = _gen_skips(1, **{k: v for k, v in gw.items()
                           if k in ("g_ipb", "g_qkv_w", "g_qkv_b", "g_ao_w",
                                    "g_ao_b", "g_ln1g", "g_ln1b", "g_ln2g",
                                    "g_ln2b", "g_f1b", "g_f2b", "g_opb")})
    skips = tuple(a and b for a, b in zip(sk0, sk1))
    center = all(skips)
    w0, _, rw0 = _prep_gen_weights(0, center, **gw)
    w1, _, rw1 = _prep_gen_weights(1, center, **gw)

    # prior MLP on host (tiny)
    pe = np.asarray(inputs["prior_emb"], np.float64)
    t = pe @ np.asarray(inputs["prior_w1"], np.float64).T \
        + np.asarray(inputs["prior_b1"], np.float64)
    t = 0.5 * t * (1.0 + np.vectorize(math.erf)(t / math.sqrt(2.0)))
    prior = (t @ np.asarray(inputs["prior_w2"], np.float64).T
             + np.asarray(inputs["prior_b2"], np.float64)).astype(np.float32)
    p_img, p_text = prior[0, :H], prior[0, H:]

    imgT = np.ascontiguousarray(image.T)
    textT = np.ascontiguousarray(text.T)

    n_pc = -(-max(len(idx1), len(idx2), 1) // GCORES)   # per-core columns
    C = max(64, -(-n_pc // 16) * 16)                    # round up to 16

    tls = _tiles(C)

    def _pack_tm(M):
        """[H, C] -> tile-major [128, NT*KC*Tt] fp8."""
        a = M.astype(mybir.dt.np(f8)).reshape(KC, 128, C).transpose(1, 0, 2)
        return np.concatenate(
            [np.ascontiguousarray(a[:, :, t0:t1]).reshape(128, -1)
             for t0, t1 in tls], axis=1)

    def shard_cols(Tsrc, Ttgt, idx):
        pad = np.zeros(GCORES * C, np.int64)
        pad[:len(idx)] = idx
        pad = pad.reshape(GCORES, C)
        return [_pack_tm(Tsrc[:, pad[c]]) for c in range(GCORES)], \
            [_pack_tm(Ttgt[:, pad[c]]) for c in range(GCORES)]

    src0, tgt0 = shard_cols(imgT, textT, idx1)
    src1, tgt1 = shard_cols(textT, imgT, idx2)

    nc = _get_program(C, skips)

    ones = np.ones((128, 128), ml_dtypes.bfloat16)
    ident = (np.eye(128, dtype=np.float32) * W8SCALE).astype(ml_dtypes.bfloat16)
    id8 = np.zeros((128, 3 * 128), mybir.dt.np(f8))
    eye8 = (np.eye(128, dtype=np.float32) * W8SCALE).astype(mybir.dt.np(f8))
    id8[:, 0:128] = eye8      # slot 0: identity (even chunk of the DR pair)
    id8[:, 256:384] = eye8    # slot 2: identity (odd chunk of the DR pair)
    in_maps = []
    for c in range(N_CORES):
        g = 0 if c < GCORES else 1
        w = w0 if g == 0 else w1
        lc = c % GCORES
        im = {
            "src": (src0 if g == 0 else src1)[lc],
            "tgt": (tgt0 if g == 0 else tgt1)[lc],
            "ip": w["ip"], "op": w["op"], "wa": w["wa"], "f1": w["f1"],
            "f2": w["f2"], "ones": ones,
        }
        if center:
            im["id8"] = id8
        else:
            im["par"] = w["par"]
            im["ident"] = ident
        in_maps.append(im)

    def assemble(results):
        def gather_out(cores, idx, rw, full):
            cols = [np.asarray(results[c]["out"])
                    .astype(np.float32)
                    .reshape(128, KC, C).transpose(1, 0, 2).reshape(H, C)
                    for c in cores]
            allc = np.concatenate(cols, axis=1)[:, :len(idx)]
            return rw * full[idx] + allc.T

        enhanced_text = text.copy()
        if len(idx1):
            enhanced_text[idx1] = gather_out(range(GCORES), idx1, rw0, text)
        enhanced_img = image.copy()
        if len(idx2):
            enhanced_img[idx2] = gather_out(range(GCORES, N_CORES), idx2,
                                            rw1, image)
        if len(idx3):
            enhanced_img[idx3] = p_img
            enhanced_text[idx3] = p_text
        return enhanced_img, enhanced_text

    return nc, in_maps, assemble


def kernel(**inputs):
    nc, in_maps, assemble = _prepare(inputs)
    res = run_bass_kernel_spmd(nc, in_maps, list(range(N_CORES)))
    return assemble(res.results)


# revision 14
# speedup vs baseline: 1.0420x; 1.0249x over previous
"""Trainium2 Bass kernel for nn_ModalGenerator (MoE-routed cross-modal generator).

Strategy (v2):
  - seq_len==1 => attention collapses to v = tgt @ wv.T; fold wv/ao_w into one
    512x512 matrix per layer (host-side) and (1-rw) into the output projection.
  - MoE routing on host: gather missing_type==1 columns (gen0) and ==2 (gen1);
    missing_type==3 rows use the tiny host-computed prior MLP.
  - Generator-split sharding: cores 0-3 run generator 0 on 1/4 of its columns
    each, cores 4-7 run generator 1. Zero collectives (host gathers/scatters).
  - All projections run in fp8-e4m3 DoubleRow mode (K=256 per pass, 2x PE
    rate): weights scaled x64, the 1/64 unscale folded into downstream scales.
  - Zero-mean-by-construction LN (requires gamma==1, beta==0, zero biases,
    which setup_inputs always produces): LN output is exactly zero-mean, and
    every pre-LN activation is (centered GEMM) + (previous LN output), so
    centering ip/wa/f2 weights over their output dim on the host makes every
    pre-LN activation zero-mean. This removes the mean stats matmuls, the
    mean subtraction, and the identity-residual matmuls of v1 entirely:
      * residual adds ride the PSUM->SBUF copy (DVE scalar_tensor_tensor)
      * LN = sumsq stat (4 bf16 ones-matmuls) -> rstd via bf16 bit-hack with
        the 1/H folded into the magic constant (reads the high half-word of
        the f32 PSUM stat directly) -> single y*rstd apply.
  - Engine balance: PE GEMMs+stats; DVE psum copies/residuals, ysq, shift;
    Pool (gpsimd) magic op + LN apply muls (SBUF only - gpsimd cannot read
    PSUM); ACT gelu + fp8 copies of LN1 outputs + final-LN fp8.
  - Wavefront software pipelining across NT=5 column tiles (<=256 wide),
    10 stages per layer + out stage; later stages issue first within a wave.
  - PSUM: 3 "mm" buffers ([128,4,256] = 2 banks each) + 2 "st" single-bank
    stat buffers = 8 banks.
  - Fallback: if the instance has nonzero biases / non-unit gamma / nonzero
    beta, use the v1 general program (kept verbatim below).
"""

import math

import numpy as np
import ml_dtypes

import concourse.bacc as bacc
import concourse.mybir as mybir
import concourse.tile as tile
from concourse.bass_utils import run_bass_kernel_spmd

f32 = mybir.dt.float32
bf16 = mybir.dt.bfloat16
f8 = mybir.dt.float8e4
i16 = mybir.dt.int16
AF = mybir.ActivationFunctionType
ALU = mybir.AluOpType
DR = mybir.MatmulPerfMode.DoubleRow

H = 512
L = 3
N_CORES = 8
GCORES = 4               # cores per generator
KC = H // 128            # 4 k-chunks of the hidden dim
FH = 4 * H               # 2048 FFN hidden
FKC = FH // 128          # 16
LN_EPS = 1e-5
MAGIC16 = 0x5F37
MAGIC16_H = 0x5F37 + 576  # folds the 1/H (H=512=2^9) into the bit-hack
W8SCALE = 64.0           # fp8 weight pre-scale

FFN_FP8 = True

# param pack column layout: [128, 128] f32 (general path only)
_P_IPB = 0
_P_LAYER = 4             # + 40*l: ba 0..3 | f1b 4..19 | f2b 20..23
#                                 | ln1g 24..27 | ln1b 28..31 | ln2g 32..35 | ln2b 36..39
_P_OPB = 124


def _pack_pcol(vec):
    """[n*128] vector -> [128, n] chunk-column layout."""
    return np.ascontiguousarray(np.asarray(vec, np.float32).reshape(-1, 128).T)


def _sb_pack(wT, dt):
    """[K, M] (K mult of 128) -> [128, (K/128)*M] SBUF chunk-major layout."""
    K, M = wT.shape
    a = np.asarray(wT, np.float32).astype(dt)
    return np.ascontiguousarray(
        a.reshape(K // 128, 128, M).transpose(1, 0, 2).reshape(128, -1))


NT_TARGET = 6            # pipeline depth (equal column tiles per core)


def _tiles(C):
    nt = min(NT_TARGET, max(1, C // 64))
    base = C // nt // 16 * 16
    sizes = [base] * nt
    extra = C - base * nt
    i = 0
    while extra > 0:
        sizes[i] += min(16, extra)
        extra -= 16
        i = (i + 1) % nt
    # small first and last tiles (short serial pipeline fill and drain),
    # big middle tiles; cap 256 so 4 psum chunks pack into 2 banks
    sizes.sort(reverse=True)
    if nt >= 3:
        ends = [sizes.pop(), sizes.pop()]
        for i in range(2):
            for _ in range(3):
                j = sizes.index(min(sizes))
                if ends[i] - 16 >= 96 and sizes[j] + 16 <= 256:
                    ends[i] -= 16
                    sizes[j] += 16
        sizes.sort(reverse=True)
        sizes = [ends[0]] + sizes + [ends[1]]
    assert max(sizes) <= 256
    ts = []
    c0 = 0
    for s in sizes:
        if s > 0:
            ts.append((c0, c0 + s))
            c0 += s
    assert c0 == C
    return ts


def _build_program_v2(C):
    """Fast path: zero biases, unit gamma, zero beta (always true for the
    harness inputs). Weights ip/wa/f2 are centered host-side."""
    nc = bacc.Bacc("TRN2", target_bir_lowering=False, debug=False,
                   num_devices=N_CORES)

    dram = {
        "src": nc.dram_tensor("src", [128, KC * C], f8, kind="ExternalInput"),
        "tgt": nc.dram_tensor("tgt", [128, KC * C], f8, kind="ExternalInput"),
        "ip": nc.dram_tensor("ip", [128, KC * H], f8, kind="ExternalInput"),
        "op": nc.dram_tensor("op", [128, KC * H], f8, kind="ExternalInput"),
        "wa": nc.dram_tensor("wa", [L, 128, KC * H], f8, kind="ExternalInput"),
        "f1": nc.dram_tensor("f1", [L, 128, KC * FH], f8, kind="ExternalInput"),
        "f2": nc.dram_tensor("f2", [L, 128, FKC * H], f8, kind="ExternalInput"),
        "ones": nc.dram_tensor("ones", [128, 128], bf16, kind="ExternalInput"),
        "id8": nc.dram_tensor("id8", [128, 3 * 128], f8, kind="ExternalInput"),
        "out": nc.dram_tensor("out", [128, KC * C], bf16, kind="ExternalOutput"),
    }
    tiles = _tiles(C)
    NT = len(tiles)
    unsc = 1.0 / W8SCALE

    with tile.TileContext(nc) as tc:
        with (
            tc.tile_pool(name="sb", bufs=2) as sb,
            tc.tile_pool(name="ps", bufs=2, space="PSUM") as psp,
        ):
            ones = sb.tile([128, 128], bf16, tag="ones", bufs=1)
            nc.sync.dma_start(ones[:], dram["ones"].ap())
            id8 = sb.tile([128, 3, 128], f8, tag="id8", bufs=1)
            nc.sync.dma_start(id8[:], dram["id8"].ap().rearrange(
                "p (a b) -> p a b", a=3))
            ipw = sb.tile([128, KC, H], f8, tag="ip", bufs=1)
            nc.sync.dma_start(ipw[:], dram["ip"].ap())
            wa0 = sb.tile([128, KC, H], f8, tag="wa", bufs=2)
            nc.sync.dma_start(wa0[:], dram["wa"].ap()[0])
            srcT = sb.tile([128, KC * C], f8, tag="src", bufs=1)
            tgtT = sb.tile([128, KC * C], f8, tag="tgt", bufs=1)
            nc.sync.dma_start(srcT[:, 0:KC * tiles[0][1]],
                              dram["src"].ap()[:, 0:KC * tiles[0][1]])
            nc.sync.dma_start(tgtT[:, 0:KC * tiles[0][1]],
                              dram["tgt"].ap()[:, 0:KC * tiles[0][1]])
            # preload the ACT gelu/copy table during the DMA fill
            wact = sb.tile([128, 128], bf16, tag="wact", bufs=1)
            nc.scalar.activation(wact[:], ones[:], AF.Gelu)
            for ti in range(1, NT):
                c0, c1 = tiles[ti]
                nc.sync.dma_start(srcT[:, KC * c0:KC * c1],
                                  dram["src"].ap()[:, KC * c0:KC * c1])
                nc.sync.dma_start(tgtT[:, KC * c0:KC * c1],
                                  dram["tgt"].ap()[:, KC * c0:KC * c1])

            def _tm(flat, ti, p):
                c0, c1 = tiles[ti]
                Ct = c1 - c0
                sl = flat[:, KC * c0 + 2 * p * Ct:KC * c0 + (2 * p + 2) * Ct]
                return sl.rearrange("q (a b) -> q a b", a=2)

            was, f1s, f2s = [wa0], [], []
            for l in range(L):
                if l > 0:
                    wa = sb.tile([128, KC, H], f8, tag="wa", bufs=2)
                    nc.sync.dma_start(wa[:], dram["wa"].ap()[l])
                    was.append(wa)
                f1w = sb.tile([128, KC, FH], f8, tag="f1", bufs=2)
                nc.sync.dma_start(f1w[:], dram["f1"].ap()[l])
                f1s.append(f1w)
                f2w = sb.tile([128, FKC, H], f8, tag="f2", bufs=2)
                nc.sync.dma_start(f2w[:], dram["f2"].ap()[l])
                f2s.append(f2w)
            opw = sb.tile([128, KC, H], f8, tag="op", bufs=1)
            nc.sync.dma_start(opw[:], dram["op"].ap())

            # per-layer activation tensors (full C width, per-tile writes)
            xn_all = []          # ln2_out bf16 per layer (attn residual)
            xf8_all = []         # ln1_out fp8 (f1 input + f2 residual);
            #                      [L-1] reused for op input
            for l in range(L):
                xn_b = sb.tile([128, KC, C], bf16, tag="x", bufs=2)
                xn_all.append((None, xn_b))
                xf8_t = sb.tile([128, KC, C], f8, tag="xf8", bufs=2)
                xf8_all.append(xf8_t)

            ps_all = [[None] * NT for _ in range(L)]   # attn psum
            pf_all = [[None] * NT for _ in range(L)]   # f2 psum
            y1_all = [[None] * NT for _ in range(L)]
            y2_all = [[None] * NT for _ in range(L)]
            st1_all = [[None] * NT for _ in range(L)]
            st2_all = [[None] * NT for _ in range(L)]
            rs1_all = [[None] * NT for _ in range(L)]
            rs2_all = [[None] * NT for _ in range(L)]
            hh_all = [[None] * NT for _ in range(L)]

            def _attn_mm(l, ti):
                wa = was[l]
                c0, c1 = tiles[ti]
                Ct = c1 - c0
                ps = psp.tile([128, 4, 256], f32, tag="mm", bufs=3)
                ps_all[l][ti] = ps
                for j in range(KC):
                    first = (j % 2 == 0)
                    last = (j % 2 == 1)
                    if l == 0:
                        for p in range(KC // 2):
                            nc.tensor.matmul(
                                ps[:, j, 0:Ct],
                                ipw[:, 2 * p:2 * p + 2, 128 * j:128 * (j + 1)],
                                _tm(srcT, ti, p),
                                start=(p == 0 and first), stop=False,
                                perf_mode=DR, skip_group_check=True)
                        for p in range(KC // 2):
                            nc.tensor.matmul(
                                ps[:, j, 0:Ct],
                                wa[:, 2 * p:2 * p + 2, 128 * j:128 * (j + 1)],
                                _tm(tgtT, ti, p),
                                start=False,
                                stop=(p == KC // 2 - 1 and last),
                                perf_mode=DR, skip_group_check=True)
                    else:
                        for p in range(KC // 2):
                            nc.tensor.matmul(
                                ps[:, j, 0:Ct],
                                wa[:, 2 * p:2 * p + 2, 128 * j:128 * (j + 1)],
                                _tm(tgtT, ti, p),
                                start=(p == 0 and first),
                                stop=(p == KC // 2 - 1 and last),
                                perf_mode=DR, skip_group_check=True)

            def _y1(l, ti):
                c0, c1 = tiles[ti]
                Ct = c1 - c0
                ps = ps_all[l][ti]
                y1 = sb.tile([128, KC, 256], bf16, tag="y", bufs=2 * NT)
                y1_all[l][ti] = y1
                if l == 0:
                    nc.vector.tensor_scalar(y1[:, :, 0:Ct], ps[:, :, 0:Ct],
                                            unsc, None, ALU.mult)
                else:
                    xp = xn_all[l - 1][1]
                    nc.vector.scalar_tensor_tensor(
                        y1[:, :, 0:Ct], ps[:, :, 0:Ct], unsc,
                        xp[:, :, c0:c1], ALU.mult, ALU.add)

            def _sq(y, Ct, holder, l, ti):
                """ysq -> sumsq stat -> shifted high half-words (the stat is
                consumed here so its PSUM slot frees within the stage)."""
                ysq = sb.tile([128, KC, 256], bf16, tag="ysq", bufs=3)
                nc.vector.tensor_mul(ysq[:, :, 0:Ct], y[:, :, 0:Ct],
                                     y[:, :, 0:Ct])
                st = psp.tile([128, 512], f32, tag="st", bufs=2)
                for k in range(KC):
                    nc.tensor.matmul(st[:, 0:Ct], ones[:], ysq[:, k, 0:Ct],
                                     start=(k == 0), stop=(k == KC - 1),
                                     skip_group_check=True)
                sh = sb.tile([128, 256], i16, tag="sh", bufs=4)
                st16 = st.bitcast(i16)          # [128, 1024]
                nc.vector.tensor_scalar(sh[:, 0:Ct], st16[:, 1:2 * Ct:2],
                                        1, None, ALU.logical_shift_right)
                holder[l][ti] = sh

            def _rstd(sh, Ct):
                """rstd = magic - (bits >> 1), 1/H folded into the magic."""
                rstd = sb.tile([128, 256], bf16, tag="rstd", bufs=4)
                nc.vector.tensor_scalar(rstd[:, 0:Ct].bitcast(i16),
                                        sh[:, 0:Ct], -1, MAGIC16_H,
                                        ALU.mult, ALU.add)
                return rstd

            def s1_stage(l, ti):
                _attn_mm(l, ti)

            def s1b_stage(l, ti):
                _y1(l, ti)

            def s2_stage(l, ti):
                c0, c1 = tiles[ti]
                _sq(y1_all[l][ti], c1 - c0, st1_all, l, ti)

            def s3_stage(l, ti):
                c0, c1 = tiles[ti]
                Ct = c1 - c0
                y1 = y1_all[l][ti]
                rstd = _rstd(st1_all[l][ti], Ct)
                rs1_all[l][ti] = rstd
                r4 = rstd[:, 0:Ct].unsqueeze(1).broadcast_to((128, KC, Ct))
                nc.gpsimd.tensor_mul(xf8_all[l][:, :, c0:c1],
                                     y1[:, :, 0:Ct], r4)

            def f1a_stage(l, ti):
                _f1_half(l, ti, 0)

            def _f1_half(l, ti, half):
                f1w = f1s[l]
                xin = xf8_all[l]
                c0, c1 = tiles[ti]
                Ct = c1 - c0
                if half == 0:
                    hh = sb.tile([128, FKC, 512], f8, tag="h", bufs=2)
                    hh_all[l][ti] = hh
                hh = hh_all[l][ti]
                for g in range(2 * half, 2 * half + 2):
                    ps = psp.tile([128, 4, 256], f32, tag="mm", bufs=3)
                    for j in range(KC):
                        mi = KC * g + j
                        first = (j % 2 == 0)
                        last = (j % 2 == 1)
                        for p in range(KC // 2):
                            nc.tensor.matmul(
                                ps[:, j, 0:Ct],
                                f1w[:, 2 * p:2 * p + 2,
                                    128 * mi:128 * (mi + 1)],
                                xin[:, 2 * p:2 * p + 2, c0:c1],
                                start=(p == 0 and first),
                                stop=(p == KC // 2 - 1 and last),
                                perf_mode=DR, skip_group_check=True)
                    nc.scalar.activation(
                        hh[:, KC * g:KC * g + KC, 0:Ct], ps[:, :, 0:Ct],
                        AF.Gelu, scale=unsc)

            def f1b_stage(l, ti):
                _f1_half(l, ti, 1)

            def f2_stage(l, ti):
                f2w = f2s[l]
                hh = hh_all[l][ti]
                xf8 = xf8_all[l]
                c0, c1 = tiles[ti]
                Ct = c1 - c0
                ps = psp.tile([128, 4, 256], f32, tag="mm", bufs=3)
                pf_all[l][ti] = ps
                for j in range(KC):
                    first = (j % 2 == 0)
                    last = (j % 2 == 1)
                    for p in range(FKC // 2):
                        nc.tensor.matmul(
                            ps[:, j, 0:Ct],
                            f2w[:, 2 * p:2 * p + 2, 128 * j:128 * (j + 1)],
                            hh[:, 2 * p:2 * p + 2, 0:Ct],
                            start=(p == 0 and first), stop=False,
                            perf_mode=DR, skip_group_check=True)
                    # residual: 64*I (fp8 DR) on the LN1 fp8 output
                    pr = j // 2
                    nc.tensor.matmul(
                        ps[:, j, 0:Ct],
                        id8[:, (0 if j % 2 == 0 else 1):(2 if j % 2 == 0 else 3), :],
                        xf8[:, 2 * pr:2 * pr + 2, c0:c1],
                        start=False, stop=last,
                        perf_mode=DR, skip_group_check=True)

            def y2_stage(l, ti):
                c0, c1 = tiles[ti]
                Ct = c1 - c0
                ps = pf_all[l][ti]
                y2 = sb.tile([128, KC, 256], bf16, tag="y", bufs=2 * NT)
                y2_all[l][ti] = y2
                nc.vector.tensor_scalar(y2[:, :, 0:Ct], ps[:, :, 0:Ct],
                                        unsc, None, ALU.mult)

            def sq2_stage(l, ti):
                c0, c1 = tiles[ti]
                _sq(y2_all[l][ti], c1 - c0, st2_all, l, ti)

            def ap2_stage(l, ti):
                c0, c1 = tiles[ti]
                Ct = c1 - c0
                y2 = y2_all[l][ti]
                rstd = _rstd(st2_all[l][ti], Ct)
                rs2_all[l][ti] = rstd
                r4 = rstd[:, 0:Ct].unsqueeze(1).broadcast_to((128, KC, Ct))
                if l < L - 1:
                    xn2 = xn_all[l][1]
                    nc.gpsimd.tensor_mul(xn2[:, :, c0:c1],
                                         y2[:, :, 0:Ct], r4)
                else:
                    # final LN feeds only the output projection: fp8
                    nc.vector.tensor_mul(xf8_all[L - 1][:, :, c0:c1],
                                         y2[:, :, 0:Ct], r4)

            def out_stage(l, ti):
                xo = xf8_all[L - 1]
                c0, c1 = tiles[ti]
                Ct = c1 - c0
                ps = psp.tile([128, 4, 256], f32, tag="mm", bufs=3)
                for j in range(KC):
                    first = (j % 2 == 0)
                    last = (j % 2 == 1)
                    for p in range(KC // 2):
                        nc.tensor.matmul(
                            ps[:, j, 0:Ct],
                            opw[:, 2 * p:2 * p + 2, 128 * j:128 * (j + 1)],
                            xo[:, 2 * p:2 * p + 2, c0:c1],
                            start=(p == 0 and first),
                            stop=(p == KC // 2 - 1 and last),
                            perf_mode=DR, skip_group_check=True)
                ot = sb.tile([128, KC, 512], bf16, tag="o", bufs=2)
                nc.scalar.activation(ot[:, :, 0:Ct], ps[:, :, 0:Ct],
                                     AF.Copy, scale=unsc)
                oap = dram["out"].ap().rearrange("p (a b) -> p a b", a=KC)
                nc.sync.dma_start(oap[:, :, c0:c1], ot[:, :, 0:Ct])

            # wavefront issue: stage ls of tile t at wave ls + t
            stage_fns = []
            for l in range(L):
                stage_fns += [
                    (s1_stage, l), (s1b_stage, l), (s2_stage, l),
                    (s3_stage, l), (f1a_stage, l), (f1b_stage, l),
                    (f2_stage, l), (y2_stage, l), (sq2_stage, l),
                    (ap2_stage, l),
                ]
            stage_fns.append((out_stage, L - 1))
            NS = len(stage_fns)
            # tiles spaced SPREAD waves apart: the active stage window spans
            # SPREAD*NT stages, covering a whole layer cycle with NT=5
            SPREAD = 2
            for wave in range(NS + SPREAD * (NT - 1) + 1):
                for t in range(0, NT):
                    s = wave - SPREAD * t
                    if 0 <= s < NS:
                        fn, l = stage_fns[s]
                        fn(l, t)

    nc.compile()
    return nc


def _build_program_general(C, skips):
    """v1 general program (biases / gamma / beta supported). Verbatim from
    the baseline kernel."""
    zb, ug, zbeta = skips
    f8dt = f8 if FFN_FP8 else bf16
    nc = bacc.Bacc("TRN2", target_bir_lowering=False, debug=False,
                   num_devices=N_CORES)

    dram = {
        "src": nc.dram_tensor("src", [128, KC * C], f8, kind="ExternalInput"),
        "tgt": nc.dram_tensor("tgt", [128, KC * C], f8, kind="ExternalInput"),
        "ip": nc.dram_tensor("ip", [128, KC * H], f8, kind="ExternalInput"),
        "op": nc.dram_tensor("op", [128, KC * H], f8, kind="ExternalInput"),
        "wa": nc.dram_tensor("wa", [L, 128, KC * H], f8, kind="ExternalInput"),
        "f1": nc.dram_tensor("f1", [L, 128, KC * FH], f8dt, kind="ExternalInput"),
        "f2": nc.dram_tensor("f2", [L, 128, FKC * H], f8dt, kind="ExternalInput"),
        "par": nc.dram_tensor("par", [128, 128], f32, kind="ExternalInput"),
        "ones": nc.dram_tensor("ones", [128, 128], bf16, kind="ExternalInput"),
        "ident": nc.dram_tensor("ident", [128, 128], bf16, kind="ExternalInput"),
        "out": nc.dram_tensor("out", [128, KC * C], bf16, kind="ExternalOutput"),
    }
    tiles = _tiles(C)
    NT = len(tiles)
    unsc = 1.0 / W8SCALE if FFN_FP8 else 1.0

    with tile.TileContext(nc) as tc:
        with (
            tc.tile_pool(name="sb", bufs=2) as sb,
            tc.tile_pool(name="ps", bufs=2, space="PSUM") as psp,
        ):
            ipw = sb.tile([128, KC, H], f8, tag="ip", bufs=1)
            nc.sync.dma_start(ipw[:], dram["ip"].ap())
            wa0 = sb.tile([128, KC, H], f8, tag="wa", bufs=2)
            nc.sync.dma_start(wa0[:], dram["wa"].ap()[0])
            srcT = sb.tile([128, KC * C], f8, tag="src", bufs=1)
            tgtT = sb.tile([128, KC * C], f8, tag="tgt", bufs=1)
            nc.sync.dma_start(srcT[:, 0:KC * tiles[0][1]],
                              dram["src"].ap()[:, 0:KC * tiles[0][1]])
            nc.sync.dma_start(tgtT[:, 0:KC * tiles[0][1]],
                              dram["tgt"].ap()[:, 0:KC * tiles[0][1]])
            ones = sb.tile([128, 128], bf16, tag="ones", bufs=1)
            nc.sync.dma_start(ones[:], dram["ones"].ap())
            ident = sb.tile([128, 128], bf16, tag="ident", bufs=1)
            nc.sync.dma_start(ident[:], dram["ident"].ap())
            for ti in range(1, NT):
                c0, c1 = tiles[ti]
                nc.sync.dma_start(srcT[:, KC * c0:KC * c1],
                                  dram["src"].ap()[:, KC * c0:KC * c1])
                nc.sync.dma_start(tgtT[:, KC * c0:KC * c1],
                                  dram["tgt"].ap()[:, KC * c0:KC * c1])

            def _tm(flat, ti, p):
                c0, c1 = tiles[ti]
                Ct = c1 - c0
                sl = flat[:, KC * c0 + 2 * p * Ct:KC * c0 + (2 * p + 2) * Ct]
                return sl.rearrange("q (a b) -> q a b", a=2)
            par = sb.tile([128, 128], f32, tag="par", bufs=1)
            nc.sync.dma_start(par[:], dram["par"].ap())

            def stats_stage(y, Ct):
                ysq = sb.tile([128, KC, 512], bf16, tag="ysq", bufs=2)
                nc.vector.tensor_mul(ysq[:, :, 0:Ct], y[:, :, 0:Ct],
                                     y[:, :, 0:Ct])
                st = psp.tile([128, 2, 256], f32, tag="st", bufs=2)
                for k in range(KC):
                    nc.tensor.matmul(st[:, 0, 0:Ct], ones[:], y[:, k, 0:Ct],
                                     start=(k == 0), stop=False,
                                     skip_group_check=True)
                for k in range(KC):
                    nc.tensor.matmul(st[:, 1, 0:Ct], ones[:], ysq[:, k, 0:Ct],
                                     start=False, stop=(k == KC - 1),
                                     skip_group_check=True)
                return st

            def ln_chain(y, st, Ct, gcol, bcol, xn, xf8, t, newton=True):
                c0, c1 = t
                m = sb.tile([128, 512], bf16, tag="m", bufs=2)
                nc.vector.tensor_scalar(m[:, 0:Ct], st[:, 0, 0:Ct], 1.0 / H,
                                        None, ALU.mult)
                msq = sb.tile([128, 512], bf16, tag="msq", bufs=2)
                nc.vector.scalar_tensor_tensor(msq[:, 0:Ct], st[:, 0, 0:Ct],
                                               1.0 / H, m[:, 0:Ct],
                                               ALU.mult, ALU.mult)
                z = sb.tile([128, 512], bf16, tag="z", bufs=2)
                nc.vector.scalar_tensor_tensor(z[:, 0:Ct], st[:, 1, 0:Ct],
                                               1.0 / H, msq[:, 0:Ct],
                                               ALU.mult, ALU.subtract)
                r = sb.tile([128, 512], bf16, tag="rx", bufs=2)
                nc.vector.tensor_scalar(r[:, 0:Ct].bitcast(i16),
                                        z[:, 0:Ct].bitcast(i16), 1, None,
                                        ALU.logical_shift_right)
                rstd = sb.tile([128, 512], bf16, tag="rstd", bufs=4)
                nc.vector.tensor_scalar(rstd[:, 0:Ct].bitcast(i16),
                                        r[:, 0:Ct].bitcast(i16), -1,
                                        MAGIC16, ALU.mult, ALU.add)
                if newton:
                    u = sb.tile([128, 512], bf16, tag="u", bufs=2)
                    nc.vector.tensor_mul(u[:, 0:Ct], rstd[:, 0:Ct],
                                         rstd[:, 0:Ct])
                    w = sb.tile([128, 512], bf16, tag="w", bufs=2)
                    nc.vector.scalar_tensor_tensor(w[:, 0:Ct], u[:, 0:Ct],
                                                   -0.5, z[:, 0:Ct],
                                                   ALU.mult, ALU.mult)
                    rstd2 = sb.tile([128, 512], bf16, tag="rstd", bufs=4)
                    nc.vector.scalar_tensor_tensor(rstd2[:, 0:Ct], w[:, 0:Ct],
                                                   1.5, rstd[:, 0:Ct],
                                                   ALU.add, ALU.mult)
                else:
                    rstd2 = rstd
                m4 = m[:, 0:Ct].unsqueeze(1).broadcast_to((128, KC, Ct))
                r4 = rstd2[:, 0:Ct].unsqueeze(1).broadcast_to((128, KC, Ct))
                if ug and zbeta:
                    u1 = sb.tile([128, KC, 512], bf16, tag="u1", bufs=2)
                    nc.vector.tensor_sub(u1[:, :, 0:Ct], y[:, :, 0:Ct], m4)
                    if xf8 is not None:
                        nc.gpsimd.tensor_mul(xf8[:, :, c0:c1],
                                             u1[:, :, 0:Ct], r4)
                        nc.vector.tensor_mul(xn[:, :, c0:c1],
                                             u1[:, :, 0:Ct], r4)
                    else:
                        nc.vector.tensor_mul(xn[:, :, c0:c1],
                                             u1[:, :, 0:Ct], r4)
                else:
                    for mm in range(KC):
                        u1 = sb.tile([128, 512], bf16, tag="u1c", bufs=1)
                        nc.vector.tensor_sub(u1[:, 0:Ct], y[:, mm, 0:Ct],
                                             m[:, 0:Ct])
                        u2 = sb.tile([128, 512], bf16, tag="u2c", bufs=1)
                        nc.vector.scalar_tensor_tensor(
                            u2[:, 0:Ct], u1[:, 0:Ct],
                            par[:, gcol + mm:gcol + mm + 1], rstd2[:, 0:Ct],
                            ALU.mult, ALU.mult)
                        nc.vector.tensor_scalar(
                            xn[:, mm, c0:c1], u2[:, 0:Ct],
                            par[:, bcol + mm:bcol + mm + 1], None, ALU.add)
                    if xf8 is not None:
                        nc.scalar.activation(xf8[:, :, c0:c1], xn[:, :, c0:c1],
                                             AF.Copy)

            x = None
            xn_all, xf8_all = [], []
            for l in range(L):
                xn_a = sb.tile([128, KC, C], bf16, tag="x", bufs=3)
                xn_b = sb.tile([128, KC, C], bf16, tag="x", bufs=3)
                xn_all.append((xn_a, xn_b))
                if FFN_FP8:
                    xf8_t = sb.tile([128, KC, C], f8, tag="xf8", bufs=2)
                    xf8_all.append(xf8_t)
                else:
                    xf8_all.append(None)

            was, f1s, f2s = [wa0], [], []
            for l in range(L):
                if l > 0:
                    wa = sb.tile([128, KC, H], f8, tag="wa", bufs=2)
                    nc.sync.dma_start(wa[:], dram["wa"].ap()[l])
                    was.append(wa)
                f1w = sb.tile([128, KC, FH], f8dt, tag="f1", bufs=2)
                nc.sync.dma_start(f1w[:], dram["f1"].ap()[l])
                f1s.append(f1w)
                f2w = sb.tile([128, FKC, H], f8dt, tag="f2", bufs=2)
                nc.sync.dma_start(f2w[:], dram["f2"].ap()[l])
                f2s.append(f2w)
            opw = sb.tile([128, KC, H], f8, tag="op", bufs=1)
            nc.sync.dma_start(opw[:], dram["op"].ap())

            ys_all = [[None] * NT for _ in range(L)]
            y2s_all = [[None] * NT for _ in range(L)]
            hh_all = [[None] * NT for _ in range(L)]

            def attn_stage(l, ti):
                pb = _P_LAYER + 40 * l
                wa = was[l]
                xp = xn_all[l - 1][1] if l > 0 else None
                c0, c1 = tiles[ti]
                Ct = c1 - c0
                y = sb.tile([128, KC, 512], bf16, tag="y", bufs=NT)
                ps = psp.tile([128, 4, 256], f32, tag="mm", bufs=3)
                for j in range(KC):
                    first = (j % 2 == 0)
                    last = (j % 2 == 1)
                    if l == 0:
                        for p in range(KC // 2):
                            nc.tensor.matmul(
                                ps[:, j, 0:Ct],
                                ipw[:, 2 * p:2 * p + 2, 128 * j:128 * (j + 1)],
                                _tm(srcT, ti, p),
                                start=(p == 0 and first), stop=False,
                                perf_mode=DR, skip_group_check=True)
                        for p in range(KC // 2):
                            nc.tensor.matmul(
                                ps[:, j, 0:Ct],
                                wa[:, 2 * p:2 * p + 2, 128 * j:128 * (j + 1)],
                                _tm(tgtT, ti, p),
                                start=False,
                                stop=(p == KC // 2 - 1 and last),
                                perf_mode=DR, skip_group_check=True)
                    else:
                        for p in range(KC // 2):
                            nc.tensor.matmul(
                                ps[:, j, 0:Ct],
                                wa[:, 2 * p:2 * p + 2, 128 * j:128 * (j + 1)],
                                _tm(tgtT, ti, p),
                                start=(p == 0 and first), stop=False,
                                perf_mode=DR, skip_group_check=True)
                        nc.tensor.matmul(
                            ps[:, j, 0:Ct], ident[:], xp[:, j, c0:c1],
                            start=False, stop=last, skip_group_check=True)
                if zb:
                    nc.scalar.activation(y[:, :, 0:Ct], ps[:, :, 0:Ct],
                                         AF.Copy, scale=unsc)
                else:
                    for j in range(KC):
                        nc.scalar.activation(
                            y[:, j, 0:Ct], ps[:, j, 0:Ct], AF.Copy,
                            scale=unsc, bias=par[:, pb + j:pb + j + 1])
                ys_all[l][ti] = y

            def ln1_stage(l, ti):
                pb = _P_LAYER + 40 * l
                t = tiles[ti]
                st = stats_stage(ys_all[l][ti], t[1] - t[0])
                ln_chain(ys_all[l][ti], st, t[1] - t[0], pb + 24, pb + 28,
                         xn_all[l][0], xf8_all[l], t, newton=False)

            def _f1_half(l, ti, half):
                pb = _P_LAYER + 40 * l
                f1w = f1s[l]
                xin = xf8_all[l] if FFN_FP8 else xn_all[l][0]
                c0, c1 = tiles[ti]
                Ct = c1 - c0
                f8dt_ = f8 if FFN_FP8 else bf16
                if half == 0:
                    hh = sb.tile([128, FKC, 512], f8dt_, tag="h", bufs=2)
                    hh_all[l][ti] = hh
                hh = hh_all[l][ti]
                for g in range(2 * half, 2 * half + 2):
                    ps = psp.tile([128, 4, 256], f32, tag="mm", bufs=3)
                    for j in range(KC):
                        mi = KC * g + j
                        first = (j % 2 == 0)
                        last = (j % 2 == 1)
                        for p in range(KC // 2):
                            nc.tensor.matmul(
                                ps[:, j, 0:Ct],
                                f1w[:, 2 * p:2 * p + 2,
                                    128 * mi:128 * (mi + 1)],
                                xin[:, 2 * p:2 * p + 2, c0:c1],
                                start=(p == 0 and first),
                                stop=(p == KC // 2 - 1 and last),
                                perf_mode=DR, skip_group_check=True)
                    if zb:
                        nc.scalar.activation(
                            hh[:, KC * g:KC * g + KC, 0:Ct], ps[:, :, 0:Ct],
                            AF.Gelu, scale=unsc)
                    else:
                        for j in range(KC):
                            mi = KC * g + j
                            nc.scalar.activation(
                                hh[:, mi, 0:Ct], ps[:, j, 0:Ct], AF.Gelu,
                                scale=unsc,
                                bias=par[:, pb + 4 + mi:pb + 4 + mi + 1])

            def f1a_stage(l, ti):
                _f1_half(l, ti, 0)

            def f1b_stage(l, ti):
                _f1_half(l, ti, 1)

            def f2_stage(l, ti):
                pb = _P_LAYER + 40 * l
                f2w = f2s[l]
                xn = xn_all[l][0]
                hh = hh_all[l][ti]
                c0, c1 = tiles[ti]
                Ct = c1 - c0
                y2 = sb.tile([128, KC, 512], bf16, tag="y", bufs=NT)
                ps = psp.tile([128, 4, 256], f32, tag="mm", bufs=3)
                for j in range(KC):
                    first = (j % 2 == 0)
                    last = (j % 2 == 1)
                    for p in range(FKC // 2):
                        nc.tensor.matmul(
                            ps[:, j, 0:Ct],
                            f2w[:, 2 * p:2 * p + 2, 128 * j:128 * (j + 1)],
                            hh[:, 2 * p:2 * p + 2, 0:Ct],
                            start=(p == 0 and first), stop=False,
                            perf_mode=DR, skip_group_check=True)
                    nc.tensor.matmul(
                        ps[:, j, 0:Ct], ident[:], xn[:, j, c0:c1],
                        start=False, stop=last, skip_group_check=True)
                if zb:
                    nc.scalar.activation(y2[:, :, 0:Ct], ps[:, :, 0:Ct],
                                         AF.Copy, scale=unsc)
                else:
                    for j in range(KC):
                        nc.scalar.activation(
                            y2[:, j, 0:Ct], ps[:, j, 0:Ct], AF.Copy,
                            scale=unsc,
                            bias=par[:, pb + 20 + j:pb + 20 + j + 1])
                y2s_all[l][ti] = y2

            def ln2_stage(l, ti):
                pb = _P_LAYER + 40 * l
                t = tiles[ti]
                st2 = stats_stage(y2s_all[l][ti], t[1] - t[0])
                ln_chain(y2s_all[l][ti], st2, t[1] - t[0], pb + 32, pb + 36,
                         xn_all[l][1],
                         xf8_all[l] if l == L - 1 else None, t,
                         newton=False)

            def out_stage(l, ti):
                xo = xf8_all[L - 1]
                c0, c1 = tiles[ti]
                Ct = c1 - c0
                ps = psp.tile([128, 4, 256], f32, tag="mm", bufs=3)
                for j in range(KC):
                    first = (j % 2 == 0)
                    last = (j % 2 == 1)
                    for p in range(KC // 2):
                        nc.tensor.matmul(
                            ps[:, j, 0:Ct],
                            opw[:, 2 * p:2 * p + 2, 128 * j:128 * (j + 1)],
                            xo[:, 2 * p:2 * p + 2, c0:c1],
                            start=(p == 0 and first),
                            stop=(p == KC // 2 - 1 and last),
                            perf_mode=DR, skip_group_check=True)
                ot = sb.tile([128, KC, 512], bf16, tag="o", bufs=2)
                if zb:
                    nc.scalar.activation(ot[:, :, 0:Ct], ps[:, :, 0:Ct],
                                         AF.Copy, scale=unsc)
                else:
                    for j in range(KC):
                        nc.scalar.activation(
                            ot[:, j, 0:Ct], ps[:, j, 0:Ct], AF.Copy,
                            scale=unsc,
                            bias=par[:, _P_OPB + j:_P_OPB + j + 1])
                oap = dram["out"].ap().rearrange("p (a b) -> p a b", a=KC)
                nc.sync.dma_start(oap[:, :, c0:c1], ot[:, :, 0:Ct])

            stage_fns = []
            for l in range(L):
                stage_fns += [
                    (attn_stage, l), (ln1_stage, l), (f1a_stage, l),
                    (f1b_stage, l), (f2_stage, l), (ln2_stage, l),
                ]
            stage_fns.append((out_stage, L - 1))
            NS = len(stage_fns)
            for wave in range(NS + NT - 1):
                for ls in range(NS - 1, -1, -1):
                    ti = wave - ls
                    if 0 <= ti < NT:
                        fn, l = stage_fns[ls]
                        fn(l, ti)

    nc.compile()
    return nc


_CACHE = {}


def _get_program(C, skips):
    fast = all(skips)
    key = (C, skips, fast)
    if key not in _CACHE:
        if fast:
            _CACHE[key] = _build_program_v2(C)
        else:
            _CACHE[key] = _build_program_general(C, skips)
    return _CACHE[key]


def _center(wT):
    """Center [K, M] weight over the output dim M so column sums of the
    produced activation vanish."""
    return wT - wT.mean(axis=1, keepdims=True)


def _prep_gen_weights(i, center, g_ipw, g_ipb, g_qkv_w, g_qkv_b, g_ao_w,
                      g_ao_b, g_ln1g, g_ln1b, g_ln2g, g_ln2b, g_f1w, g_f1b,
                      g_f2w, g_f2b, g_opw, g_opb, g_rw):
    wa, ba = [], []
    for l in range(L):
        _wq, _wk, wv = np.split(g_qkv_w[i, l], 3, axis=0)
        _bq, _bk, bv = np.split(g_qkv_b[i, l], 3)
        wa.append((g_ao_w[i, l] @ wv).T)                 # [K=H, M=H]
        ba.append(g_ao_b[i, l] + bv @ g_ao_w[i, l].T)
    rw = float(g_rw[i])
    ws = W8SCALE if FFN_FP8 else 1.0
    f8np = mybir.dt.np(f8 if FFN_FP8 else bf16)
    ipT = g_ipw[i].T.astype(np.float64)
    waT = [wa[l].astype(np.float64) for l in range(L)]
    f2T = [g_f2w[i, l].T.astype(np.float64) for l in range(L)]
    if center:
        ipT = _center(ipT)
        waT = [_center(w) for w in waT]
        f2T = [_center(w) for w in f2T]
    ipP = _sb_pack(W8SCALE * ipT, mybir.dt.np(f8))
    opP = _sb_pack(W8SCALE * (1.0 - rw) * g_opw[i].T, mybir.dt.np(f8))
    waP = np.stack([_sb_pack(W8SCALE * waT[l], mybir.dt.np(f8))
                    for l in range(L)])
    f1P = np.stack([_sb_pack(ws * g_f1w[i, l].T, f8np) for l in range(L)])
    f2P = np.stack([_sb_pack(ws * f2T[l], f8np) for l in range(L)])

    par = np.zeros((128, 128), np.float32)
    par[:, _P_IPB:_P_IPB + KC] = _pack_pcol(g_ipb[i])
    for l in range(L):
        pb = _P_LAYER + 40 * l
        bal = ba[l] + (g_ipb[i] if l == 0 else 0.0)   # layer-0 fuses ipb
        par[:, pb:pb + 4] = _pack_pcol(bal)
        par[:, pb + 4:pb + 20] = _pack_pcol(g_f1b[i, l])
        par[:, pb + 20:pb + 24] = _pack_pcol(g_f2b[i, l])
        par[:, pb + 24:pb + 28] = _pack_pcol(g_ln1g[i, l])
        par[:, pb + 28:pb + 32] = _pack_pcol(g_ln1b[i, l])
        par[:, pb + 32:pb + 36] = _pack_pcol(g_ln2g[i, l])
        par[:, pb + 36:pb + 40] = _pack_pcol(g_ln2b[i, l])
    par[:, _P_OPB:_P_OPB + KC] = _pack_pcol((1.0 - rw) * g_opb[i])

    zb = bool(np.all(g_ipb[i] == 0) and all(np.all(b == 0) for b in ba)
              and np.all(g_f1b[i] == 0) and np.all(g_f2b[i] == 0)
              and np.all(g_opb[i] == 0))
    ug = bool(np.all(g_ln1g[i] == 1) and np.all(g_ln2g[i] == 1))
    zbeta = bool(np.all(g_ln1b[i] == 0) and np.all(g_ln2b[i] == 0))
    return {"ip": ipP, "op": opP, "wa": waP, "f1": f1P, "f2": f2P,
            "par": par}, (zb, ug, zbeta), rw


def _gen_skips(i, g_ipb, g_qkv_w, g_qkv_b, g_ao_w, g_ao_b, g_ln1g, g_ln1b,
               g_ln2g, g_ln2b, g_f1b, g_f2b, g_opb, **_):
    ba = []
    for l in range(L):
        _bq, _bk, bv = np.split(g_qkv_b[i, l], 3)
        ba.append(g_ao_b[i, l] + bv @ g_ao_w[i, l].T)
    zb = bool(np.all(g_ipb[i] == 0) and all(np.all(b == 0) for b in ba)
              and np.all(g_f1b[i] == 0) and np.all(g_f2b[i] == 0)
              and np.all(g_opb[i] == 0))
    ug = bool(np.all(g_ln1g[i] == 1) and np.all(g_ln2g[i] == 1))
    zbeta = bool(np.all(g_ln1b[i] == 0) and np.all(g_ln2b[i] == 0))
    return (zb, ug, zbeta)


def _prepare(inputs):
    """Host-side prep. Returns (nc, in_maps, assemble)."""
    image = np.asarray(inputs["image_features"], np.float32)
    text = np.asarray(inputs["text_features"], np.float32)
    mt = np.asarray(inputs["missing_type"])

    idx1 = np.nonzero(mt == 1)[0]      # gen0 (img -> text) fills text
    idx2 = np.nonzero(mt == 2)[0]      # gen1 (text -> img) fills img
    idx3 = np.nonzero(mt == 3)[0]

    gw = {k: np.asarray(v) for k, v in inputs.items() if k.startswith("g_")}
    sk0 = _gen_skips(0, **{k: v for k, v in gw.items()
                           if k in ("g_ipb", "g_qkv_w", "g_qkv_b", "g_ao_w",
                                    "g_ao_b", "g_ln1g", "g_ln1b", "g_ln2g",
                                    "g_ln2b", "g_f1b", "g_f2b", "g_opb")})
    sk1 = _gen_skips(1, **{k: v for k, v in gw.items()
                           if k in ("g_ipb", "g_qkv_w", "g_qkv_b", "g_ao_w",
                                    "g_ao_b", "g_ln1g", "g_ln1b", "g_ln2g",
                                    "g_ln2b", "g_f1b", "g_f2b", "g_opb")})
    skips = tuple(a and b for a, b in zip(sk0, sk1))
    center = all(skips)
    w0, _, rw0 = _prep_gen_weights(0, center, **gw)
    w1, _, rw1 = _prep_gen_weights(1, center, **gw)

    # prior MLP on host (tiny)
    pe = np.asarray(inputs["prior_emb"], np.float64)
    t = pe @ np.asarray(inputs["prior_w1"], np.float64).T \
        + np.asarray(inputs["prior_b1"], np.float64)
    t = 0.5 * t * (1.0 + np.vectorize(math.erf)(t / math.sqrt(2.0)))
    prior = (t @ np.asarray(inputs["prior_w2"], np.float64).T
             + np.asarray(inputs["prior_b2"], np.float64)).astype(np.float32)
    p_img, p_text = prior[0, :H], prior[0, H:]

    imgT = np.ascontiguousarray(image.T)
    textT = np.ascontiguousarray(text.T)

    n_pc = -(-max(len(idx1), len(idx2), 1) // GCORES)   # per-core columns
    C = max(64, -(-n_pc // 16) * 16)                    # round up to 16

    tls = _tiles(C)

    def _pack_tm(M):
        """[H, C] -> tile-major [128, NT*KC*Tt] fp8."""
        a = M.astype(mybir.dt.np(f8)).reshape(KC, 128, C).transpose(1, 0, 2)
        return np.concatenate(
            [np.ascontiguousarray(a[:, :, t0:t1]).reshape(128, -1)
             for t0, t1 in tls], axis=1)

    def shard_cols(Tsrc, Ttgt, idx):
        pad = np.zeros(GCORES * C, np.int64)
        pad[:len(idx)] = idx
        pad = pad.reshape(GCORES, C)
        return [_pack_tm(Tsrc[:, pad[c]]) for c in range(GCORES)], \
            [_pack_tm(Ttgt[:, pad[c]]) for c in range(GCORES)]

    src0, tgt0 = shard_cols(imgT, textT, idx1)
    src1, tgt1 = shard_cols(textT, imgT, idx2)

    nc = _get_program(C, skips)

    ones = np.ones((128, 128), ml_dtypes.bfloat16)
    ident = (np.eye(128, dtype=np.float32) * W8SCALE).astype(ml_dtypes.bfloat16)
    id8 = np.zeros((128, 3 * 128), mybir.dt.np(f8))
    eye8 = (np.eye(128, dtype=np.float32) * W8SCALE).astype(mybir.dt.np(f8))
    id8[:, 0:128] = eye8      # slot 0: identity (even chunk of the DR pair)
    id8[:, 256:384] = eye8    # slot 2: identity (odd chunk of the DR pair)
    in_maps = []
    for c in range(N_CORES):
        g = 0 if c < GCORES else 1
        w = w0 if g == 0 else w1
        lc = c % GCORES
        im = {
            "src": (src0 if g == 0 else src1)[lc],
            "tgt": (tgt0 if g == 0 else tgt1)[lc],
            "ip": w["ip"], "op": w["op"], "wa": w["wa"], "f1": w["f1"],
            "f2": w["f2"], "ones": ones,
        }
        if center:
            im["id8"] = id8
        else:
            im["par"] = w["par"]
            im["ident"] = ident
        in_maps.append(im)

    def assemble(results):
        def gather_out(cores, idx, rw, full):
            cols = [np.asarray(results[c]["out"])
                    .astype(np.float32)
                    .reshape(128, KC, C).transpose(1, 0, 2).reshape(H, C)
                    for c in cores]
            allc = np.concatenate(cols, axis=1)[:, :len(idx)]
            return rw * full[idx] + allc.T

        enhanced_text = text.copy()
        if len(idx1):
            enhanced_text[idx1] = gather_out(range(GCORES), idx1, rw0, text)
        enhanced_img = image.copy()
        if len(idx2):
            enhanced_img[idx2] = gather_out(range(GCORES, N_CORES), idx2,
                                            rw1, image)
        if len(idx3):
            enhanced_img[idx3] = p_img
            enhanced_text[idx3] = p_text
        return enhanced_img, enhanced_text

    return nc, in_maps, assemble


def kernel(**inputs):
    nc, in_maps, assemble = _prepare(inputs)
    res = run_bass_kernel_spmd(nc, in_maps, list(range(N_CORES)))
    return assemble(res.results)
